# revision 3
# baseline (speedup 1.0000x reference)
"""Trainium2 Bass kernel for the ExoplanetGNN heterograph message-passing net.

Self-contained: builds host-side edge grids, compiles one SPMD Bass program,
runs it on 8 NeuronCores via run_bass_kernel_spmd, reassembles the output.

Design:
 - dst-sharded edges: core c owns planet shard c and star shard c and all edges
   whose dst lands there; aggregation is complete per core (no reduce).
 - node feature tables (bf16, node-major [rows, 64]) are fully replicated per
   core; after each layer, shards are AllGathered into the next layer's table.
 - per-128-edge tile: indirect-DMA row gather ([128,1] offsets), DVE one-hot
   (iota is_equal dst_rel) * (1/deg), TensorE segment matmul accumulating
   transposed aggregates [64 feat x 128 nodes] in PSUM.
 - per 512-node chunk: stacked [128, 512] rhs (two relations' aggregates for
   planets / aggregate+xT for stars), one or two K=128/64 matmuls apply the
   SAGE linear layers, ScalarE fuses bias+ReLU, HWDGE dma-transpose produces
   node-major tiles for the table shard; a feat-major copy (xT) is kept for
   the next layer's self term.
 - layer 2 skips the star update and fuses the readout MLP on the planet path.
"""

import math

import numpy as np
import ml_dtypes

import concourse.bass as bass
import concourse.bacc as bacc
import concourse.mybir as mybir
import concourse.tile as tile
from concourse.bass import IndirectOffsetOnAxis
from concourse.bass_utils import run_bass_kernel_spmd

BF16 = ml_dtypes.bfloat16
BF = mybir.dt.bfloat16
F32 = mybir.dt.float32
I32 = mybir.dt.int32
AF = mybir.ActivationFunctionType
ALU = mybir.AluOpType

C = 8          # cores
N_SWDGE_Q = 4
BLK = 128      # dst nodes per block
CHUNK_BLKS = 4 # node blocks per compute chunk
SPAN_COLS = 512  # max index columns per span load
MSG_TAGS = 16  # distinct per-gather msg buffer tags
MSG_BUFS = 3   # ring depth per tag (48 gathers in flight)




def _patch_indirect_queue():
    """Recompile BassGpSimd.indirect_dma_start with a queue= parameter."""
    import inspect, textwrap, re
    src_ = textwrap.dedent(inspect.getsource(bass.BassGpSimd.indirect_dma_start))
    src_ = src_.replace("def indirect_dma_start(", "def indirect_dma_start_q(")
    src_ = src_.replace("compute_op: mybir.AluOpType = mybir.AluOpType.bypass,",
                        "compute_op: mybir.AluOpType = mybir.AluOpType.bypass, queue: str = \"qPoolDynamic\",")
    src_ = src_.replace('queue="qPoolDynamic"', "queue=queue")
    ns = vars(bass).copy()
    exec(compile(src_, "<indirect_q>", "exec"), ns)
    bass.BassGpSimd.indirect_dma_start_q = ns["indirect_dma_start_q"]


_patch_indirect_queue()


class Cfg:
    def __init__(self, np_=500000, ns_=200000, fp=32, fs=16, h=64, l=3):
        self.NP, self.NS, self.FP, self.FS, self.H, self.L = np_, ns_, fp, fs, h, l
        assert np_ % C == 0 and ns_ % C == 0
        self.SP, self.SS = np_ // C, ns_ // C
        self.PB = -(-self.SP // BLK)
        self.SB = -(-self.SS // BLK)
        self.NPP, self.NSP = self.PB * BLK, self.SB * BLK
        self.NPT, self.NST = C * self.NPP, C * self.NSP


def _prep_rel(src, dst, src_shard, src_pad, dst_shard, dst_blocks, seg=512):
    """Build per-core tile-transposed edge arrays for one relation.

    Returns (srcT [C,128,T] int32, dr [C,128,T] f32, w [C,128,T] f32,
    tiles_per_block list[int] of len dst_blocks).
    Slot (t, p) holds edge i = <pos p of tile t>; tile t belongs to one dst
    block; pad slots: src=0, dr=-1, w=0.
    """
    src = np.asarray(src, np.int64)
    dst = np.asarray(dst, np.int64)
    core = dst // dst_shard
    loc = dst - core * dst_shard
    blk = loc // seg
    rel = loc - blk * seg
    PB = -(-(dst_blocks * BLK) // seg)
    key = core * PB + blk
    cnt = np.bincount(key, minlength=C * PB).reshape(C, PB)
    tpb = np.maximum(1, -(-cnt.max(axis=0) // BLK))
    tile_base = np.concatenate([[0], np.cumsum(tpb)]).astype(np.int64)
    T = int(tile_base[-1])

    order = np.argsort(key, kind="stable")
    key_s = key[order]
    firsts = np.searchsorted(key_s, np.arange(C * PB))
    pos = np.arange(len(key_s)) - firsts[key_s]
    blk_s = blk[order]
    t_idx = tile_base[blk_s] + pos // BLK
    p_idx = pos % BLK
    c_idx = core[order]

    deg = np.bincount(core * dst_shard + loc, minlength=C * dst_shard).astype(np.float64)
    w_e = (1.0 / np.maximum(deg, 1.0))[core * dst_shard + loc]

    s_core = src // src_shard
    s_pad = s_core * src_pad + (src - s_core * src_shard)

    srcT = np.zeros((C, BLK, T), np.int32)
    dr = np.full((C, BLK, T), -1.0, np.float32)
    w = np.zeros((C, BLK, T), np.float32)
    srcT[c_idx, p_idx, t_idx] = s_pad[order]
    dr[c_idx, p_idx, t_idx] = rel[order]
    w[c_idx, p_idx, t_idx] = w_e[order]
    return srcT, dr, w, tpb.tolist()


def _chunks(nblocks):
    out = []
    b = 0
    while b < nblocks:
        nb = min(CHUNK_BLKS, nblocks - b)
        out.append((b, nb))
        b += nb
    return out


def _spans(tpb, chunks):
    """Partition the tile axis into spans of <= SPAN_COLS cols aligned to
    chunk boundaries. Returns (tile_base, spans list[(t0,t1)], span_of_chunk)."""
    tile_base = [0]
    for t in tpb:
        tile_base.append(tile_base[-1] + t)
    spans = []
    span_of_chunk = []
    cur0 = 0
    for (b, nb) in chunks:
        t0, t1 = tile_base[b], tile_base[b + nb]
        if t1 - cur0 > SPAN_COLS and t0 > cur0:
            spans.append((cur0, t0))
            cur0 = t0
        span_of_chunk.append(len(spans))
    spans.append((cur0, tile_base[-1]))
    return tile_base, spans, span_of_chunk


def build(cfg, grids, b2val):
    """grids: dict rel -> (tile_base, tpb, spans, span_of_chunk, T_total)."""
    H, FP, FS, L = cfg.H, cfg.FP, cfg.FS, cfg.L
    nc = bacc.Bacc(None, target_bir_lowering=False, num_devices=C, num_swdge_queues=4)

    def param(name, shape, dt):
        return nc.declare_dram_parameter(name, shape, dt, isOutput=False)

    xpt = param("xpt", [FP, cfg.NPP], BF)
    xst = param("xst", [FS, cfg.NSP], BF)
    eparams = {}
    for r in ("orb", "hst", "sib"):
        T = grids[r][4]
        eparams[r] = (
            param(f"{r}_src", [BLK, T], I32),
            param(f"{r}_dr", [BLK, T], F32),
            param(f"{r}_w", [BLK, T], F32),
        )
    iota_p = param("iota", [128, 512], F32)
    wp_p = param("wp", [FP, H], BF)
    bp_p = param("bp", [H, 1], F32)
    ws_p = param("ws", [FS, H], BF)
    bs_p = param("bs", [H, 1], F32)
    wstack_s_p = param("wstack_s", [L, 2 * H, H], BF)
    wstack_p_p = param("wstack_p", [L, 2 * H, H], BF)
    wr_p_p = param("wr_p", [L, H, H], BF)
    bias_s_p = param("bias_s", [L, H, 1], F32)
    bias_p_p = param("bias_p", [L, H, 1], F32)
    w1_p = param("w1", [H, H // 2], BF)
    b1_p = param("b1", [H // 2, 1], F32)
    w2_p = param("w2", [H // 2, 1], BF)
    out_p = nc.declare_dram_parameter("out", [1, cfg.NPP], F32, isOutput=True)

    pchunks = _chunks(cfg.PB)
    schunks = _chunks(cfg.SB)

    with tile.TileContext(nc) as tc:
        with (
            tc.tile_pool(name="const", bufs=1) as cp,
            tc.tile_pool(name="dram", bufs=1, space="DRAM") as dp,
            tc.tile_pool(name="idx", bufs=2) as ip,
            tc.tile_pool(name="msg", bufs=3) as mp,
            tc.tile_pool(name="sel", bufs=16) as selp,
            tc.tile_pool(name="work", bufs=4) as wkp,
            tc.tile_pool(name="psum", bufs=1, space="PSUM") as pp,
        ):
            # ---- persistent DRAM state ----
            hp_tab = [
                dp.tile([cfg.NPT, H], BF, addr_space="Shared", tag=f"hp_tab{i}", name=f"hp_tab{i}")
                for i in range(L)
            ]
            hs_tab = [
                dp.tile([cfg.NST, H], BF, addr_space="Shared", tag=f"hs_tab{i}", name=f"hs_tab{i}")
                for i in range(L)
            ]
            xpT = [dp.tile([H, cfg.NPP], BF, tag=f"xpT{i}", name=f"xpT{i}") for i in range(2)]
            xsT = [dp.tile([H, cfg.NSP], BF, tag=f"xsT{i}", name=f"xsT{i}") for i in range(2)]
            hp_shard = dp.tile([cfg.NPP, H], BF, tag="hp_shard")
            hs_shard = dp.tile([cfg.NSP, H], BF, tag="hs_shard")

            # ---- consts ----
            iota_t = cp.tile([128, 512], F32, tag="iota")
            nc.sync.dma_start(out=iota_t[:], in_=iota_p[:, :])
            wp_t = cp.tile([FP, H], BF, tag="wp")
            nc.sync.dma_start(out=wp_t[:], in_=wp_p[:, :])
            ws_t = cp.tile([FS, H], BF, tag="ws")
            nc.sync.dma_start(out=ws_t[:], in_=ws_p[:, :])
            bp_t = cp.tile([H, 1], F32, tag="bp")
            nc.sync.dma_start(out=bp_t[:], in_=bp_p[:, :])
            bs_t = cp.tile([H, 1], F32, tag="bs")
            nc.sync.dma_start(out=bs_t[:], in_=bs_p[:, :])
            w1_t = cp.tile([H, H // 2], BF, tag="w1")
            nc.sync.dma_start(out=w1_t[:], in_=w1_p[:, :])
            b1_t = cp.tile([H // 2, 1], F32, tag="b1")
            nc.sync.dma_start(out=b1_t[:], in_=b1_p[:, :])
            w2_t = cp.tile([H // 2, 1], BF, tag="w2")
            nc.sync.dma_start(out=w2_t[:], in_=w2_p[:, :])
            wstack_s_t, wstack_p_t, wr_p_t, bias_s_t, bias_p_t = [], [], [], [], []
            for l in range(L):
                t = cp.tile([2 * H, H], BF, tag=f"wss{l}")
                nc.sync.dma_start(out=t[:], in_=wstack_s_p[l, :, :])
                wstack_s_t.append(t)
                t = cp.tile([2 * H, H], BF, tag=f"wsp{l}")
                nc.sync.dma_start(out=t[:], in_=wstack_p_p[l, :, :])
                wstack_p_t.append(t)
                t = cp.tile([H, H], BF, tag=f"wrp{l}")
                nc.sync.dma_start(out=t[:], in_=wr_p_p[l, :, :])
                wr_p_t.append(t)
                t = cp.tile([H, 1], F32, tag=f"bss{l}")
                nc.sync.dma_start(out=t[:], in_=bias_s_p[l, :, :])
                bias_s_t.append(t)
                t = cp.tile([H, 1], F32, tag=f"bsp{l}")
                nc.sync.dma_start(out=t[:], in_=bias_p_p[l, :, :])
                bias_p_t.append(t)

            def allgather(shard, tab):
                nc.gpsimd.collective_compute(
                    "AllGather",
                    ALU.bypass,
                    replica_groups=[list(range(C))],
                    ins=[shard[:, :]],
                    outs=[tab[:, :]],
                )

            def write_out_chunk(ob, c0, cw, nb, shard, xT_next):
                nc.sync.dma_start(out=xT_next[:, c0 : c0 + cw], in_=ob[:, :cw])
                for bi in range(nb):
                    nm = wkp.tile([128, H], BF, tag="nm", bufs=6)
                    nc.sync.dma_start_transpose(
                        out=nm[:], in_=ob[:, bi * 128 : (bi + 1) * 128]
                    )
                    r0 = c0 + bi * 128
                    nc.sync.dma_start(out=shard[r0 : r0 + 128, :], in_=nm[:])

            class SpanState:
                def __init__(self, rel):
                    self.rel = rel
                    self.cur = -1
                    self.tiles = None

                def ensure(self, si, spans):
                    if self.cur == si:
                        return
                    self.cur = si
                    t0, t1 = spans[si]
                    n = t1 - t0
                    sp, dp_, wp_ = eparams[self.rel]
                    st = ip.tile([BLK, n], I32, tag=f"{self.rel}_src")
                    nc.sync.dma_start(out=st[:], in_=sp[:, t0:t1])
                    dt_ = ip.tile([BLK, n], F32, tag=f"{self.rel}_dr")
                    nc.sync.dma_start(out=dt_[:], in_=dp_[:, t0:t1])
                    wt = ip.tile([BLK, n], F32, tag=f"{self.rel}_w")
                    nc.sync.dma_start(out=wt[:], in_=wp_[:, t0:t1])
                    self.tiles = (st, dt_, wt, t0)

            def agg_chunk(rel, state, table, b0, nb, ci, agg_psum):
                """Emit gathers + one-hot + segment matmuls for chunk [b0, b0+nb)
                of relation rel, accumulating aggT into agg_psum [64, nb*128].

                Each gather lands in its own small tile so the per-tile matmul
                only waits on its OWN gather (not the whole chunk's), keeping
                the Pool engine (the serialized descgen bottleneck) saturated."""
                tile_base, tpb, spans, soc, T = grids[rel]
                cw = nb * BLK
                state.ensure(soc[ci], spans)
                st, dt_, wt, t0 = state.tiles
                c_t0, c_t1 = tile_base[ci], tile_base[ci + 1]
                gk = c_t1 - c_t0
                msgs = []
                for j in range(gk):
                    t = c_t0 + j
                    jj = t - t0
                    qn = t % N_SWDGE_Q
                    m = mp.tile([128, H], BF, tag=f"{rel}_msg{j % MSG_TAGS}",
                                bufs=MSG_BUFS, name=f"{rel}_msgc")
                    nc.gpsimd.indirect_dma_start_q(
                        out=m[:],
                        out_offset=None,
                        in_=table[:, :],
                        in_offset=IndirectOffsetOnAxis(
                            ap=st[:, jj : jj + 1], axis=0
                        ),
                        queue=f"qPoolDynamic{qn or ''}",
                    )
                    msgs.append(m)
                for j in range(gk):
                    t = c_t0 + j
                    jj = t - t0
                    sel = selp.tile([128, cw], BF, tag="sel", name="sel")
                    nc.vector.tensor_scalar(
                        out=sel[:],
                        in0=iota_t[:, :cw],
                        scalar1=dt_[:, jj : jj + 1],
                        scalar2=wt[:, jj : jj + 1],
                        op0=ALU.is_equal,
                        op1=ALU.mult,
                    )
                    nc.tensor.matmul(
                        out=agg_psum[:, :cw],
                        lhsT=msgs[j][:],
                        rhs=sel[:],
                        start=(j == 0),
                        stop=(j == gk - 1),
                    )

            # =================== input projection ===================
            for (b0, nb) in pchunks:
                cw = nb * BLK
                c0 = b0 * BLK
                xp = wkp.tile([FP, cw], BF, tag="xp")
                nc.sync.dma_start(out=xp[:], in_=xpt[:, c0 : c0 + cw])
                po = pp.tile([H, cw], F32, tag="out", bufs=2)
                nc.tensor.matmul(
                    out=po[:], lhsT=wp_t[:], rhs=xp[:], start=True, stop=True
                )
                ob = wkp.tile([H, cw], BF, tag="ob")
                nc.scalar.activation(
                    out=ob[:], in_=po[:], func=AF.Relu, bias=bp_t[:], scale=1.0
                )
                write_out_chunk(ob, c0, cw, nb, hp_shard, xpT[0])
            for (b0, nb) in schunks:
                cw = nb * BLK
                c0 = b0 * BLK
                xs = wkp.tile([FS, cw], BF, tag="xs")
                nc.sync.dma_start(out=xs[:], in_=xst[:, c0 : c0 + cw])
                po = pp.tile([H, cw], F32, tag="out", bufs=2)
                nc.tensor.matmul(
                    out=po[:], lhsT=ws_t[:], rhs=xs[:], start=True, stop=True
                )
                ob = wkp.tile([H, cw], BF, tag="ob")
                nc.scalar.activation(
                    out=ob[:], in_=po[:], func=AF.Relu, bias=bs_t[:], scale=1.0
                )
                write_out_chunk(ob, c0, cw, nb, hs_shard, xsT[0])
            allgather(hp_shard, hp_tab[0])
            allgather(hs_shard, hs_tab[0])

            # =================== SAGE layers ===================
            for l in range(L):
                rp, wpar = l % 2, (l + 1) % 2
                rv, wv = l, l + 1
                # ---- stars (skip at last layer: no consumer) ----
                if l < L - 1:
                    st_orb = SpanState("orb")
                    for ci, (b0, nb) in enumerate(schunks):
                        cw = nb * BLK
                        c0 = b0 * BLK
                        agg = pp.tile([H, CHUNK_BLKS * BLK], F32, tag="agg_a", bufs=2)
                        agg_chunk("orb", st_orb, hp_tab[rv], b0, nb, ci, agg)
                        stacked = wkp.tile([2 * H, cw], BF, tag="stacked")
                        nc.scalar.activation(
                            out=stacked[0:H, :], in_=agg[:, :cw], func=AF.Copy
                        )
                        nc.sync.dma_start(
                            out=stacked[H : 2 * H, :], in_=xsT[rp][:, c0 : c0 + cw]
                        )
                        po = pp.tile([H, cw], F32, tag="out", bufs=2)
                        nc.tensor.matmul(
                            out=po[:],
                            lhsT=wstack_s_t[l][:],
                            rhs=stacked[:],
                            start=True,
                            stop=True,
                        )
                        ob = wkp.tile([H, cw], BF, tag="ob")
                        nc.scalar.activation(
                            out=ob[:], in_=po[:], func=AF.Relu,
                            bias=bias_s_t[l][:], scale=1.0,
                        )
                        write_out_chunk(ob, c0, cw, nb, hs_shard, xsT[wpar])
                # ---- planets ----
                st_hst = SpanState("hst")
                st_sib = SpanState("sib")
                for ci, (b0, nb) in enumerate(pchunks):
                    cw = nb * BLK
                    c0 = b0 * BLK
                    agg_h = pp.tile([H, CHUNK_BLKS * BLK], F32, tag="agg_a", bufs=2)
                    agg_chunk("hst", st_hst, hs_tab[rv], b0, nb, ci, agg_h)
                    agg_s = pp.tile([H, CHUNK_BLKS * BLK], F32, tag="agg_b", bufs=2)
                    agg_chunk("sib", st_sib, hp_tab[rv], b0, nb, ci, agg_s)
                    stacked = wkp.tile([2 * H, cw], BF, tag="stacked")
                    nc.scalar.activation(
                        out=stacked[0:H, :], in_=agg_h[:, :cw], func=AF.Copy
                    )
                    nc.scalar.activation(
                        out=stacked[H : 2 * H, :], in_=agg_s[:, :cw], func=AF.Copy
                    )
                    xt = wkp.tile([H, cw], BF, tag="xt")
                    nc.sync.dma_start(out=xt[:], in_=xpT[rp][:, c0 : c0 + cw])
                    po = pp.tile([H, cw], F32, tag="out", bufs=2)
                    nc.tensor.matmul(
                        out=po[:],
                        lhsT=wstack_p_t[l][:],
                        rhs=stacked[:],
                        start=True,
                        stop=False,
                    )
                    nc.tensor.matmul(
                        out=po[:], lhsT=wr_p_t[l][:], rhs=xt[:],
                        start=False, stop=True,
                    )
                    ob = wkp.tile([H, cw], BF, tag="ob")
                    nc.scalar.activation(
                        out=ob[:], in_=po[:], func=AF.Relu,
                        bias=bias_p_t[l][:], scale=1.0,
                    )
                    if l < L - 1:
                        write_out_chunk(ob, c0, cw, nb, hp_shard, xpT[wpar])
                    else:
                        # fused readout: relu(ob^T W1 + b1) W2 + b2
                        pr = pp.tile([H // 2, cw], F32, tag="r1", bufs=1)
                        nc.tensor.matmul(
                            out=pr[:], lhsT=w1_t[:], rhs=ob[:], start=True, stop=True
                        )
                        r1 = wkp.tile([H // 2, cw], BF, tag="r1sb")
                        nc.scalar.activation(
                            out=r1[:], in_=pr[:], func=AF.Relu,
                            bias=b1_t[:], scale=1.0,
                        )
                        py = pp.tile([1, cw], F32, tag="y", bufs=1)
                        nc.tensor.matmul(
                            out=py[:], lhsT=w2_t[:], rhs=r1[:], start=True, stop=True
                        )
                        ysb = wkp.tile([1, cw], F32, tag="ysb")
                        nc.vector.tensor_scalar_add(
                            out=ysb[:], in0=py[:], scalar1=float(b2val)
                        )
                        nc.sync.dma_start(out=out_p[0:1, c0 : c0 + cw], in_=ysb[:])
                if l < L - 1:
                    allgather(hp_shard, hp_tab[wv])
                    allgather(hs_shard, hs_tab[wv])

    nc.finalize()
    return nc


def _prep_all(inputs, cfg):
    f32 = np.float32
    xp = np.asarray(inputs["x_planet"], f32)
    xs = np.asarray(inputs["x_star"], f32)
    Wp = np.asarray(inputs["Wp"], f32)
    bp = np.asarray(inputs["bp"], f32)
    Ws = np.asarray(inputs["Ws"], f32)
    bs = np.asarray(inputs["bs"], f32)
    Wl = np.asarray(inputs["Wl"], f32)
    bl = np.asarray(inputs["bl"], f32)
    Wr = np.asarray(inputs["Wr"], f32)
    W1 = np.asarray(inputs["W1"], f32)
    b1 = np.asarray(inputs["b1"], f32)
    W2 = np.asarray(inputs["W2"], f32)
    b2 = np.asarray(inputs["b2"], f32)

    orb = _prep_rel(inputs["orbits_src"], inputs["orbits_dst"],
                    cfg.SP, cfg.NPP, cfg.SS, cfg.SB)
    hst = _prep_rel(inputs["hosts_src"], inputs["hosts_dst"],
                    cfg.SS, cfg.NSP, cfg.SP, cfg.PB)
    sib = _prep_rel(inputs["sib_src"], inputs["sib_dst"],
                    cfg.SP, cfg.NPP, cfg.SP, cfg.PB)

    grids = {}
    for name, r in (("orb", orb), ("hst", hst), ("sib", sib)):
        tpb = r[3]
        nblocks = cfg.SB if name == "orb" else cfg.PB
        nsegs = len(tpb)
        seg_chunks = [(i, 1) for i in range(nsegs)]
        tile_base, spans, soc = _spans(tpb, seg_chunks)
        grids[name] = (tile_base, tpb, spans, soc, tile_base[-1])

    L, H = cfg.L, cfg.H
    wstack_s = np.stack([np.concatenate([Wl[l, 0], Wr[l, 0]], 0) for l in range(L)])
    wstack_p = np.stack(
        [np.concatenate([0.5 * Wl[l, 1], 0.5 * Wl[l, 2]], 0) for l in range(L)]
    )
    wr_p = np.stack([0.5 * (Wr[l, 1] + Wr[l, 2]) for l in range(L)])
    bias_s = np.stack([bl[l, 0][:, None] for l in range(L)])
    bias_p = np.stack([0.5 * (bl[l, 1] + bl[l, 2])[:, None] for l in range(L)])
    iota = np.tile(np.arange(512, dtype=np.float32), (128, 1))

    common = {
        "iota": iota,
        "wp": Wp.astype(BF16), "bp": bp[:, None],
        "ws": Ws.astype(BF16), "bs": bs[:, None],
        "wstack_s": wstack_s.astype(BF16), "wstack_p": wstack_p.astype(BF16),
        "wr_p": wr_p.astype(BF16),
        "bias_s": bias_s, "bias_p": bias_p,
        "w1": W1.astype(BF16), "b1": b1[:, None], "w2": W2.astype(BF16),
    }
    in_maps = []
    for c in range(C):
        xpt_c = np.zeros((cfg.FP, cfg.NPP), BF16)
        xpt_c[:, : cfg.SP] = xp[c * cfg.SP : (c + 1) * cfg.SP].T.astype(BF16)
        xst_c = np.zeros((cfg.FS, cfg.NSP), BF16)
        xst_c[:, : cfg.SS] = xs[c * cfg.SS : (c + 1) * cfg.SS].T.astype(BF16)
        m = dict(common)
        m["xpt"] = xpt_c
        m["xst"] = xst_c
        for name, r in (("orb", orb), ("hst", hst), ("sib", sib)):
            m[f"{name}_src"] = r[0][c]
            m[f"{name}_dr"] = r[1][c]
            m[f"{name}_w"] = r[2][c]
        in_maps.append(m)
    return in_maps, grids, float(b2[0])


LAST_RESULT = None


def kernel(_cfg=None, _trace=False, **inputs):
    global LAST_RESULT
    cfg = _cfg or Cfg()
    in_maps, grids, b2val = _prep_all(inputs, cfg)
    nc = build(cfg, grids, b2val)
    res = run_bass_kernel_spmd(nc, in_maps, list(range(C)), trace=_trace)
    LAST_RESULT = res
    out = np.concatenate(
        [res.results[c]["out"][0, : cfg.SP] for c in range(C)]
    ).astype(np.float32)
    return out



# revision 5
# speedup vs baseline: 1.0041x; 1.0041x over previous
"""Trainium2 Bass kernel for the ExoplanetGNN heterograph message-passing net.

Self-contained: builds host-side edge grids, compiles one SPMD Bass program,
runs it on 8 NeuronCores via run_bass_kernel_spmd, reassembles the output.

Design:
 - dst-sharded edges: core c owns planet shard c and star shard c and all edges
   whose dst lands there; aggregation is complete per core (no reduce).
 - node feature tables (bf16, node-major [rows, 64]) are fully replicated per
   core; after each layer, shards are AllGathered into the next layer's table.
 - per-128-edge tile: indirect-DMA row gather ([128,1] offsets), DVE one-hot
   (iota is_equal dst_rel) * (1/deg), TensorE segment matmul accumulating
   transposed aggregates [64 feat x 128 nodes] in PSUM.
 - per 512-node chunk: stacked [128, 512] rhs (two relations' aggregates for
   planets / aggregate+xT for stars), one or two K=128/64 matmuls apply the
   SAGE linear layers, ScalarE fuses bias+ReLU, HWDGE dma-transpose produces
   node-major tiles for the table shard; a feat-major copy (xT) is kept for
   the next layer's self term.
 - layer 2 skips the star update and fuses the readout MLP on the planet path.
"""

import math

import numpy as np
import ml_dtypes

import concourse.bass as bass
import concourse.bacc as bacc
import concourse.mybir as mybir
import concourse.tile as tile
from concourse.bass import IndirectOffsetOnAxis
from concourse.bass_utils import run_bass_kernel_spmd

BF16 = ml_dtypes.bfloat16
BF = mybir.dt.bfloat16
F32 = mybir.dt.float32
I32 = mybir.dt.int32
AF = mybir.ActivationFunctionType
ALU = mybir.AluOpType

C = 8          # cores
N_SWDGE_Q = 4
BLK = 128      # dst nodes per block
CHUNK_BLKS = 4 # node blocks per compute chunk
SPAN_COLS = 512  # max index columns per span load
MSG_TAGS = 16  # distinct per-gather msg buffer tags
MSG_BUFS = 3   # ring depth per tag (48 gathers in flight)




def _patch_indirect_queue():
    """Recompile BassGpSimd.indirect_dma_start with a queue= parameter."""
    import inspect, textwrap, re
    src_ = textwrap.dedent(inspect.getsource(bass.BassGpSimd.indirect_dma_start))
    src_ = src_.replace("def indirect_dma_start(", "def indirect_dma_start_q(")
    src_ = src_.replace("compute_op: mybir.AluOpType = mybir.AluOpType.bypass,",
                        "compute_op: mybir.AluOpType = mybir.AluOpType.bypass, queue: str = \"qPoolDynamic\",")
    src_ = src_.replace('queue="qPoolDynamic"', "queue=queue")
    ns = vars(bass).copy()
    exec(compile(src_, "<indirect_q>", "exec"), ns)
    bass.BassGpSimd.indirect_dma_start_q = ns["indirect_dma_start_q"]


_patch_indirect_queue()


class Cfg:
    def __init__(self, np_=500000, ns_=200000, fp=32, fs=16, h=64, l=3):
        self.NP, self.NS, self.FP, self.FS, self.H, self.L = np_, ns_, fp, fs, h, l
        assert np_ % C == 0 and ns_ % C == 0
        self.SP, self.SS = np_ // C, ns_ // C
        self.PB = -(-self.SP // BLK)
        self.SB = -(-self.SS // BLK)
        self.NPP, self.NSP = self.PB * BLK, self.SB * BLK
        self.NPT, self.NST = C * self.NPP, C * self.NSP


def _prep_rel(src, dst, src_shard, src_pad, dst_shard, dst_blocks, seg=512):
    """Build per-core tile-transposed edge arrays for one relation.

    Returns (srcT [C,128,T] int32, dr [C,128,T] f32, w [C,128,T] f32,
    tiles_per_block list[int] of len dst_blocks).
    Slot (t, p) holds edge i = <pos p of tile t>; tile t belongs to one dst
    block; pad slots: src=0, dr=-1, w=0.
    """
    src = np.asarray(src, np.int64)
    dst = np.asarray(dst, np.int64)
    core = dst // dst_shard
    loc = dst - core * dst_shard
    blk = loc // seg
    rel = loc - blk * seg
    PB = -(-(dst_blocks * BLK) // seg)
    key = core * PB + blk
    cnt = np.bincount(key, minlength=C * PB).reshape(C, PB)
    tpb = np.maximum(1, -(-cnt.max(axis=0) // BLK))
    tile_base = np.concatenate([[0], np.cumsum(tpb)]).astype(np.int64)
    T = int(tile_base[-1])

    order = np.argsort(key, kind="stable")
    key_s = key[order]
    firsts = np.searchsorted(key_s, np.arange(C * PB))
    pos = np.arange(len(key_s)) - firsts[key_s]
    blk_s = blk[order]
    t_idx = tile_base[blk_s] + pos // BLK
    p_idx = pos % BLK
    c_idx = core[order]

    deg = np.bincount(core * dst_shard + loc, minlength=C * dst_shard).astype(np.float64)
    w_e = (1.0 / np.maximum(deg, 1.0))[core * dst_shard + loc]

    s_core = src // src_shard
    s_pad = s_core * src_pad + (src - s_core * src_shard)

    srcT = np.zeros((C, BLK, T), np.int32)
    dr = np.full((C, BLK, T), -1.0, np.float32)
    w = np.zeros((C, BLK, T), np.float32)
    srcT[c_idx, p_idx, t_idx] = s_pad[order]
    dr[c_idx, p_idx, t_idx] = rel[order]
    w[c_idx, p_idx, t_idx] = w_e[order]
    return srcT, dr, w, tpb.tolist()


def _chunks(nblocks):
    out = []
    b = 0
    while b < nblocks:
        nb = min(CHUNK_BLKS, nblocks - b)
        out.append((b, nb))
        b += nb
    return out


def _spans(tpb, chunks):
    """Partition the tile axis into spans of <= SPAN_COLS cols aligned to
    chunk boundaries. Returns (tile_base, spans list[(t0,t1)], span_of_chunk)."""
    tile_base = [0]
    for t in tpb:
        tile_base.append(tile_base[-1] + t)
    spans = []
    span_of_chunk = []
    cur0 = 0
    for (b, nb) in chunks:
        t0, t1 = tile_base[b], tile_base[b + nb]
        if t1 - cur0 > SPAN_COLS and t0 > cur0:
            spans.append((cur0, t0))
            cur0 = t0
        span_of_chunk.append(len(spans))
    spans.append((cur0, tile_base[-1]))
    return tile_base, spans, span_of_chunk


def build(cfg, grids, b2val):
    """grids: dict rel -> (tile_base, tpb, spans, span_of_chunk, T_total)."""
    H, FP, FS, L = cfg.H, cfg.FP, cfg.FS, cfg.L
    nc = bacc.Bacc(None, target_bir_lowering=False, num_devices=C, num_swdge_queues=4)

    def param(name, shape, dt):
        return nc.declare_dram_parameter(name, shape, dt, isOutput=False)

    xpt = param("xpt", [FP, cfg.NPP], BF)
    xst = param("xst", [FS, cfg.NSP], BF)
    eparams = {}
    for r in ("orb", "hst", "sib"):
        T = grids[r][4]
        eparams[r] = (
            param(f"{r}_src", [BLK, T], I32),
            param(f"{r}_dr", [BLK, T], F32),
            param(f"{r}_w", [BLK, T], F32),
        )
    iota_p = param("iota", [128, 512], F32)
    wp_p = param("wp", [FP, H], BF)
    bp_p = param("bp", [H, 1], F32)
    ws_p = param("ws", [FS, H], BF)
    bs_p = param("bs", [H, 1], F32)
    wstack_s_p = param("wstack_s", [L, 2 * H, H], BF)
    wstack_p_p = param("wstack_p", [L, 2 * H, H], BF)
    wr_p_p = param("wr_p", [L, H, H], BF)
    bias_s_p = param("bias_s", [L, H, 1], F32)
    bias_p_p = param("bias_p", [L, H, 1], F32)
    w1_p = param("w1", [H, H // 2], BF)
    b1_p = param("b1", [H // 2, 1], F32)
    w2_p = param("w2", [H // 2, 1], BF)
    out_p = nc.declare_dram_parameter("out", [1, cfg.NPP], F32, isOutput=True)

    pchunks = _chunks(cfg.PB)
    schunks = _chunks(cfg.SB)

    with tile.TileContext(nc) as tc:
        with (
            tc.tile_pool(name="const", bufs=1) as cp,
            tc.tile_pool(name="dram", bufs=1, space="DRAM") as dp,
            tc.tile_pool(name="idx", bufs=2) as ip,
            tc.tile_pool(name="msg", bufs=3) as mp,
            tc.tile_pool(name="sel", bufs=16) as selp,
            tc.tile_pool(name="work", bufs=4) as wkp,
            tc.tile_pool(name="psum", bufs=1, space="PSUM") as pp,
        ):
            # ---- persistent DRAM state ----
            hp_tab = [
                dp.tile([cfg.NPT, H], BF, addr_space="Shared", tag=f"hp_tab{i}", name=f"hp_tab{i}")
                for i in range(L)
            ]
            hs_tab = [
                dp.tile([cfg.NST, H], BF, addr_space="Shared", tag=f"hs_tab{i}", name=f"hs_tab{i}")
                for i in range(L)
            ]
            xpT = [dp.tile([H, cfg.NPP], BF, tag=f"xpT{i}", name=f"xpT{i}") for i in range(2)]
            xsT = [dp.tile([H, cfg.NSP], BF, tag=f"xsT{i}", name=f"xsT{i}") for i in range(2)]
            hp_shard = dp.tile([cfg.NPP, H], BF, tag="hp_shard")
            hs_shard = dp.tile([cfg.NSP, H], BF, tag="hs_shard")

            # ---- consts ----
            iota_t = cp.tile([128, 512], F32, tag="iota")
            nc.sync.dma_start(out=iota_t[:], in_=iota_p[:, :])
            wp_t = cp.tile([FP, H], BF, tag="wp")
            nc.sync.dma_start(out=wp_t[:], in_=wp_p[:, :])
            ws_t = cp.tile([FS, H], BF, tag="ws")
            nc.sync.dma_start(out=ws_t[:], in_=ws_p[:, :])
            bp_t = cp.tile([H, 1], F32, tag="bp")
            nc.sync.dma_start(out=bp_t[:], in_=bp_p[:, :])
            bs_t = cp.tile([H, 1], F32, tag="bs")
            nc.sync.dma_start(out=bs_t[:], in_=bs_p[:, :])
            w1_t = cp.tile([H, H // 2], BF, tag="w1")
            nc.sync.dma_start(out=w1_t[:], in_=w1_p[:, :])
            b1_t = cp.tile([H // 2, 1], F32, tag="b1")
            nc.sync.dma_start(out=b1_t[:], in_=b1_p[:, :])
            w2_t = cp.tile([H // 2, 1], BF, tag="w2")
            nc.sync.dma_start(out=w2_t[:], in_=w2_p[:, :])
            wstack_s_t, wstack_p_t, wr_p_t, bias_s_t, bias_p_t = [], [], [], [], []
            for l in range(L):
                t = cp.tile([2 * H, H], BF, tag=f"wss{l}")
                nc.sync.dma_start(out=t[:], in_=wstack_s_p[l, :, :])
                wstack_s_t.append(t)
                t = cp.tile([2 * H, H], BF, tag=f"wsp{l}")
                nc.sync.dma_start(out=t[:], in_=wstack_p_p[l, :, :])
                wstack_p_t.append(t)
                t = cp.tile([H, H], BF, tag=f"wrp{l}")
                nc.sync.dma_start(out=t[:], in_=wr_p_p[l, :, :])
                wr_p_t.append(t)
                t = cp.tile([H, 1], F32, tag=f"bss{l}")
                nc.sync.dma_start(out=t[:], in_=bias_s_p[l, :, :])
                bias_s_t.append(t)
                t = cp.tile([H, 1], F32, tag=f"bsp{l}")
                nc.sync.dma_start(out=t[:], in_=bias_p_p[l, :, :])
                bias_p_t.append(t)

            def allgather(shard, tab):
                nc.gpsimd.collective_compute(
                    "AllGather",
                    ALU.bypass,
                    replica_groups=[list(range(C))],
                    ins=[shard[:, :]],
                    outs=[tab[:, :]],
                )

            def write_out_chunk(ob, c0, cw, nb, shard, xT_next):
                nc.sync.dma_start(out=xT_next[:, c0 : c0 + cw], in_=ob[:, :cw])
                for bi in range(nb):
                    nm = wkp.tile([128, H], BF, tag="nm", bufs=6)
                    nc.sync.dma_start_transpose(
                        out=nm[:], in_=ob[:, bi * 128 : (bi + 1) * 128]
                    )
                    r0 = c0 + bi * 128
                    nc.sync.dma_start(out=shard[r0 : r0 + 128, :], in_=nm[:])

            class SpanState:
                def __init__(self, rel):
                    self.rel = rel
                    self.cur = -1
                    self.tiles = None

                def ensure(self, si, spans):
                    if self.cur == si:
                        return
                    self.cur = si
                    t0, t1 = spans[si]
                    n = t1 - t0
                    sp, dp_, wp_ = eparams[self.rel]
                    st = ip.tile([BLK, n], I32, tag=f"{self.rel}_src")
                    nc.sync.dma_start(out=st[:], in_=sp[:, t0:t1])
                    dt_ = ip.tile([BLK, n], F32, tag=f"{self.rel}_dr")
                    nc.sync.dma_start(out=dt_[:], in_=dp_[:, t0:t1])
                    wt = ip.tile([BLK, n], F32, tag=f"{self.rel}_w")
                    nc.sync.dma_start(out=wt[:], in_=wp_[:, t0:t1])
                    self.tiles = (st, dt_, wt, t0)

            def agg_chunk(rel, state, table, b0, nb, ci, agg_psum):
                """Emit gathers + one-hot + segment matmuls for chunk [b0, b0+nb)
                of relation rel, accumulating aggT into agg_psum [64, nb*128].

                Each gather lands in its own small tile so the per-tile matmul
                only waits on its OWN gather (not the whole chunk's), keeping
                the Pool engine (the serialized descgen bottleneck) saturated."""
                tile_base, tpb, spans, soc, T = grids[rel]
                cw = nb * BLK
                state.ensure(soc[ci], spans)
                st, dt_, wt, t0 = state.tiles
                c_t0, c_t1 = tile_base[ci], tile_base[ci + 1]
                gk = c_t1 - c_t0
                msgs = []
                for j in range(gk):
                    t = c_t0 + j
                    jj = t - t0
                    qn = t % N_SWDGE_Q
                    m = mp.tile([128, H], BF, tag=f"{rel}_msg{j % MSG_TAGS}",
                                bufs=MSG_BUFS, name=f"{rel}_msgc")
                    nc.gpsimd.indirect_dma_start_q(
                        out=m[:],
                        out_offset=None,
                        in_=table[:, :],
                        in_offset=IndirectOffsetOnAxis(
                            ap=st[:, jj : jj + 1], axis=0
                        ),
                        queue=f"qPoolDynamic{qn or ''}",
                    )
                    msgs.append(m)
                for j in range(gk):
                    t = c_t0 + j
                    jj = t - t0
                    sel = selp.tile([128, cw], BF, tag="sel", name="sel")
                    nc.vector.tensor_scalar(
                        out=sel[:],
                        in0=iota_t[:, :cw],
                        scalar1=dt_[:, jj : jj + 1],
                        scalar2=wt[:, jj : jj + 1],
                        op0=ALU.is_equal,
                        op1=ALU.mult,
                    )
                    nc.tensor.matmul(
                        out=agg_psum[:, :cw],
                        lhsT=msgs[j][:],
                        rhs=sel[:],
                        start=(j == 0),
                        stop=(j == gk - 1),
                    )

            # =================== input projection ===================
            for (b0, nb) in pchunks:
                cw = nb * BLK
                c0 = b0 * BLK
                xp = wkp.tile([FP, cw], BF, tag="xp")
                nc.sync.dma_start(out=xp[:], in_=xpt[:, c0 : c0 + cw])
                po = pp.tile([H, cw], F32, tag="out", bufs=2)
                nc.tensor.matmul(
                    out=po[:], lhsT=wp_t[:], rhs=xp[:], start=True, stop=True
                )
                ob = wkp.tile([H, cw], BF, tag="ob")
                nc.scalar.activation(
                    out=ob[:], in_=po[:], func=AF.Relu, bias=bp_t[:], scale=1.0
                )
                write_out_chunk(ob, c0, cw, nb, hp_shard, xpT[0])
            for (b0, nb) in schunks:
                cw = nb * BLK
                c0 = b0 * BLK
                xs = wkp.tile([FS, cw], BF, tag="xs")
                nc.sync.dma_start(out=xs[:], in_=xst[:, c0 : c0 + cw])
                po = pp.tile([H, cw], F32, tag="out", bufs=2)
                nc.tensor.matmul(
                    out=po[:], lhsT=ws_t[:], rhs=xs[:], start=True, stop=True
                )
                ob = wkp.tile([H, cw], BF, tag="ob")
                nc.scalar.activation(
                    out=ob[:], in_=po[:], func=AF.Relu, bias=bs_t[:], scale=1.0
                )
                write_out_chunk(ob, c0, cw, nb, hs_shard, xsT[0])
            allgather(hp_shard, hp_tab[0])
            allgather(hs_shard, hs_tab[0])

            # =================== SAGE layers ===================
            # Planets first, then AG(hp) issued so it overlaps the star phase
            # (stars read layer-l tables only), then stars, then AG(hs).
            for l in range(L):
                rp, wpar = l % 2, (l + 1) % 2
                rv, wv = l, l + 1
                # ---- planets ----
                st_hst = SpanState("hst")
                st_sib = SpanState("sib")
                for ci, (b0, nb) in enumerate(pchunks):
                    cw = nb * BLK
                    c0 = b0 * BLK
                    agg_h = pp.tile([H, CHUNK_BLKS * BLK], F32, tag="agg_a", bufs=2)
                    agg_chunk("hst", st_hst, hs_tab[rv], b0, nb, ci, agg_h)
                    agg_s = pp.tile([H, CHUNK_BLKS * BLK], F32, tag="agg_b", bufs=2)
                    agg_chunk("sib", st_sib, hp_tab[rv], b0, nb, ci, agg_s)
                    stacked = wkp.tile([2 * H, cw], BF, tag="stacked")
                    nc.scalar.activation(
                        out=stacked[0:H, :], in_=agg_h[:, :cw], func=AF.Copy
                    )
                    nc.scalar.activation(
                        out=stacked[H : 2 * H, :], in_=agg_s[:, :cw], func=AF.Copy
                    )
                    xt = wkp.tile([H, cw], BF, tag="xt")
                    nc.sync.dma_start(out=xt[:], in_=xpT[rp][:, c0 : c0 + cw])
                    po = pp.tile([H, cw], F32, tag="out", bufs=2)
                    nc.tensor.matmul(
                        out=po[:],
                        lhsT=wstack_p_t[l][:],
                        rhs=stacked[:],
                        start=True,
                        stop=False,
                    )
                    nc.tensor.matmul(
                        out=po[:], lhsT=wr_p_t[l][:], rhs=xt[:],
                        start=False, stop=True,
                    )
                    ob = wkp.tile([H, cw], BF, tag="ob")
                    nc.scalar.activation(
                        out=ob[:], in_=po[:], func=AF.Relu,
                        bias=bias_p_t[l][:], scale=1.0,
                    )
                    if l < L - 1:
                        write_out_chunk(ob, c0, cw, nb, hp_shard, xpT[wpar])
                    else:
                        # fused readout: relu(ob^T W1 + b1) W2 + b2
                        pr = pp.tile([H // 2, cw], F32, tag="r1", bufs=1)
                        nc.tensor.matmul(
                            out=pr[:], lhsT=w1_t[:], rhs=ob[:], start=True, stop=True
                        )
                        r1 = wkp.tile([H // 2, cw], BF, tag="r1sb")
                        nc.scalar.activation(
                            out=r1[:], in_=pr[:], func=AF.Relu,
                            bias=b1_t[:], scale=1.0,
                        )
                        py = pp.tile([1, cw], F32, tag="y", bufs=1)
                        nc.tensor.matmul(
                            out=py[:], lhsT=w2_t[:], rhs=r1[:], start=True, stop=True
                        )
                        ysb = wkp.tile([1, cw], F32, tag="ysb")
                        nc.vector.tensor_scalar_add(
                            out=ysb[:], in0=py[:], scalar1=float(b2val)
                        )
                        nc.sync.dma_start(out=out_p[0:1, c0 : c0 + cw], in_=ysb[:])
                if l < L - 1:
                    allgather(hp_shard, hp_tab[wv])
                # ---- stars (skip at last layer: no consumer); overlaps AG(hp)
                if l < L - 1:
                    st_orb = SpanState("orb")
                    for ci, (b0, nb) in enumerate(schunks):
                        cw = nb * BLK
                        c0 = b0 * BLK
                        agg = pp.tile([H, CHUNK_BLKS * BLK], F32, tag="agg_a", bufs=2)
                        agg_chunk("orb", st_orb, hp_tab[rv], b0, nb, ci, agg)
                        stacked = wkp.tile([2 * H, cw], BF, tag="stacked")
                        nc.scalar.activation(
                            out=stacked[0:H, :], in_=agg[:, :cw], func=AF.Copy
                        )
                        nc.sync.dma_start(
                            out=stacked[H : 2 * H, :], in_=xsT[rp][:, c0 : c0 + cw]
                        )
                        po = pp.tile([H, cw], F32, tag="out", bufs=2)
                        nc.tensor.matmul(
                            out=po[:],
                            lhsT=wstack_s_t[l][:],
                            rhs=stacked[:],
                            start=True,
                            stop=True,
                        )
                        ob = wkp.tile([H, cw], BF, tag="ob")
                        nc.scalar.activation(
                            out=ob[:], in_=po[:], func=AF.Relu,
                            bias=bias_s_t[l][:], scale=1.0,
                        )
                        write_out_chunk(ob, c0, cw, nb, hs_shard, xsT[wpar])
                    allgather(hs_shard, hs_tab[wv])

    nc.finalize()
    return nc


def _prep_all(inputs, cfg):
    f32 = np.float32
    xp = np.asarray(inputs["x_planet"], f32)
    xs = np.asarray(inputs["x_star"], f32)
    Wp = np.asarray(inputs["Wp"], f32)
    bp = np.asarray(inputs["bp"], f32)
    Ws = np.asarray(inputs["Ws"], f32)
    bs = np.asarray(inputs["bs"], f32)
    Wl = np.asarray(inputs["Wl"], f32)
    bl = np.asarray(inputs["bl"], f32)
    Wr = np.asarray(inputs["Wr"], f32)
    W1 = np.asarray(inputs["W1"], f32)
    b1 = np.asarray(inputs["b1"], f32)
    W2 = np.asarray(inputs["W2"], f32)
    b2 = np.asarray(inputs["b2"], f32)

    orb = _prep_rel(inputs["orbits_src"], inputs["orbits_dst"],
                    cfg.SP, cfg.NPP, cfg.SS, cfg.SB)
    hst = _prep_rel(inputs["hosts_src"], inputs["hosts_dst"],
                    cfg.SS, cfg.NSP, cfg.SP, cfg.PB)
    sib = _prep_rel(inputs["sib_src"], inputs["sib_dst"],
                    cfg.SP, cfg.NPP, cfg.SP, cfg.PB)

    grids = {}
    for name, r in (("orb", orb), ("hst", hst), ("sib", sib)):
        tpb = r[3]
        nblocks = cfg.SB if name == "orb" else cfg.PB
        nsegs = len(tpb)
        seg_chunks = [(i, 1) for i in range(nsegs)]
        tile_base, spans, soc = _spans(tpb, seg_chunks)
        grids[name] = (tile_base, tpb, spans, soc, tile_base[-1])

    L, H = cfg.L, cfg.H
    wstack_s = np.stack([np.concatenate([Wl[l, 0], Wr[l, 0]], 0) for l in range(L)])
    wstack_p = np.stack(
        [np.concatenate([0.5 * Wl[l, 1], 0.5 * Wl[l, 2]], 0) for l in range(L)]
    )
    wr_p = np.stack([0.5 * (Wr[l, 1] + Wr[l, 2]) for l in range(L)])
    bias_s = np.stack([bl[l, 0][:, None] for l in range(L)])
    bias_p = np.stack([0.5 * (bl[l, 1] + bl[l, 2])[:, None] for l in range(L)])
    iota = np.tile(np.arange(512, dtype=np.float32), (128, 1))

    common = {
        "iota": iota,
        "wp": Wp.astype(BF16), "bp": bp[:, None],
        "ws": Ws.astype(BF16), "bs": bs[:, None],
        "wstack_s": wstack_s.astype(BF16), "wstack_p": wstack_p.astype(BF16),
        "wr_p": wr_p.astype(BF16),
        "bias_s": bias_s, "bias_p": bias_p,
        "w1": W1.astype(BF16), "b1": b1[:, None], "w2": W2.astype(BF16),
    }
    in_maps = []
    for c in range(C):
        xpt_c = np.zeros((cfg.FP, cfg.NPP), BF16)
        xpt_c[:, : cfg.SP] = xp[c * cfg.SP : (c + 1) * cfg.SP].T.astype(BF16)
        xst_c = np.zeros((cfg.FS, cfg.NSP), BF16)
        xst_c[:, : cfg.SS] = xs[c * cfg.SS : (c + 1) * cfg.SS].T.astype(BF16)
        m = dict(common)
        m["xpt"] = xpt_c
        m["xst"] = xst_c
        for name, r in (("orb", orb), ("hst", hst), ("sib", sib)):
            m[f"{name}_src"] = r[0][c]
            m[f"{name}_dr"] = r[1][c]
            m[f"{name}_w"] = r[2][c]
        in_maps.append(m)
    return in_maps, grids, float(b2[0])


LAST_RESULT = None


def kernel(_cfg=None, _trace=False, **inputs):
    global LAST_RESULT
    cfg = _cfg or Cfg()
    in_maps, grids, b2val = _prep_all(inputs, cfg)
    nc = build(cfg, grids, b2val)
    res = run_bass_kernel_spmd(nc, in_maps, list(range(C)), trace=_trace)
    LAST_RESULT = res
    out = np.concatenate(
        [res.results[c]["out"][0, : cfg.SP] for c in range(C)]
    ).astype(np.float32)
    return out



# revision 9
# speedup vs baseline: 1.4020x; 1.3962x over previous
"""Trainium2 Bass kernel for the ExoplanetGNN heterograph message-passing net.

Self-contained: builds host-side edge grids, compiles one SPMD Bass program,
runs it on 8 NeuronCores via run_bass_kernel_spmd, reassembles the output.

Design:
 - dst-sharded edges: core c owns planet shard c and star shard c and all edges
   whose dst lands there; aggregation is complete per core (no reduce).
 - node feature tables (bf16, node-major [rows, 64]) are fully replicated per
   core; after each layer, shards are AllGathered into the next layer's table.
 - per-128-edge tile: indirect-DMA row gather ([128,1] offsets), DVE one-hot
   (iota is_equal dst_rel) * (1/deg), TensorE segment matmul accumulating
   transposed aggregates [64 feat x 128 nodes] in PSUM.
 - per 512-node chunk: stacked [128, 512] rhs (two relations' aggregates for
   planets / aggregate+xT for stars), one or two K=128/64 matmuls apply the
   SAGE linear layers, ScalarE fuses bias+ReLU, HWDGE dma-transpose produces
   node-major tiles for the table shard; a feat-major copy (xT) is kept for
   the next layer's self term.
 - layer 2 skips the star update and fuses the readout MLP on the planet path.
"""

import math

import numpy as np
import ml_dtypes

import concourse.bass as bass
import concourse.bacc as bacc
import concourse.mybir as mybir
import concourse.tile as tile
from concourse.bass import IndirectOffsetOnAxis
from concourse.bass_utils import run_bass_kernel_spmd

BF16 = ml_dtypes.bfloat16
BF = mybir.dt.bfloat16
F32 = mybir.dt.float32
I32 = mybir.dt.int32
AF = mybir.ActivationFunctionType
ALU = mybir.AluOpType

C = 8          # cores
N_SWDGE_Q = 4
BLK = 128      # dst nodes per block
CHUNK_BLKS = 4 # node blocks per compute chunk
SPAN_COLS = 512  # max index columns per span load
MSG_TAGS = 16  # distinct per-gather msg buffer tags
MSG_BUFS = 3   # ring depth per tag (48 gathers in flight)




def _patch_indirect_queue():
    """Recompile BassGpSimd.indirect_dma_start with a queue= parameter."""
    import inspect, textwrap, re
    src_ = textwrap.dedent(inspect.getsource(bass.BassGpSimd.indirect_dma_start))
    src_ = src_.replace("def indirect_dma_start(", "def indirect_dma_start_q(")
    src_ = src_.replace("compute_op: mybir.AluOpType = mybir.AluOpType.bypass,",
                        "compute_op: mybir.AluOpType = mybir.AluOpType.bypass, queue: str = \"qPoolDynamic\",")
    src_ = src_.replace('queue="qPoolDynamic"', "queue=queue")
    ns = vars(bass).copy()
    exec(compile(src_, "<indirect_q>", "exec"), ns)
    bass.BassGpSimd.indirect_dma_start_q = ns["indirect_dma_start_q"]


_patch_indirect_queue()


class Cfg:
    def __init__(self, np_=500000, ns_=200000, fp=32, fs=16, h=64, l=3):
        self.NP, self.NS, self.FP, self.FS, self.H, self.L = np_, ns_, fp, fs, h, l
        assert np_ % C == 0 and ns_ % C == 0
        self.SP, self.SS = np_ // C, ns_ // C
        self.PB = -(-self.SP // BLK)
        self.SB = -(-self.SS // BLK)
        self.NPP, self.NSP = self.PB * BLK, self.SB * BLK
        self.NPT, self.NST = C * self.NPP, C * self.NSP


def _prep_rel(src, dst, src_shard, src_pad, dst_shard, dst_blocks, seg=512):
    """Build per-core tile-transposed edge arrays for one relation.

    Returns (srcT [C,128,T] int32, dr [C,128,T] f32, w [C,128,T] f32,
    tiles_per_block list[int] of len dst_blocks).
    Slot (t, p) holds edge i = <pos p of tile t>; tile t belongs to one dst
    block; pad slots: src=0, dr=-1, w=0.
    """
    src = np.asarray(src, np.int64)
    dst = np.asarray(dst, np.int64)
    core = dst // dst_shard
    loc = dst - core * dst_shard
    blk = loc // seg
    rel = loc - blk * seg
    PB = -(-(dst_blocks * BLK) // seg)
    key = core * PB + blk
    cnt = np.bincount(key, minlength=C * PB).reshape(C, PB)
    tpb = np.maximum(1, -(-cnt.max(axis=0) // BLK))
    tile_base = np.concatenate([[0], np.cumsum(tpb)]).astype(np.int64)
    T = int(tile_base[-1])

    order = np.argsort(key, kind="stable")
    key_s = key[order]
    firsts = np.searchsorted(key_s, np.arange(C * PB))
    pos = np.arange(len(key_s)) - firsts[key_s]
    blk_s = blk[order]
    t_idx = tile_base[blk_s] + pos // BLK
    p_idx = pos % BLK
    c_idx = core[order]

    deg = np.bincount(core * dst_shard + loc, minlength=C * dst_shard).astype(np.float64)
    w_e = (1.0 / np.maximum(deg, 1.0))[core * dst_shard + loc]

    s_core = src // src_shard
    s_pad = s_core * src_pad + (src - s_core * src_shard)

    srcT = np.zeros((C, BLK, T), np.int32)
    dr = np.full((C, BLK, T), -1.0, np.float32)
    w = np.zeros((C, BLK, T), np.float32)
    srcT[c_idx, p_idx, t_idx] = s_pad[order]
    dr[c_idx, p_idx, t_idx] = rel[order]
    w[c_idx, p_idx, t_idx] = w_e[order]
    return srcT, dr, w, tpb.tolist()


def _chunks(nblocks):
    out = []
    b = 0
    while b < nblocks:
        nb = min(CHUNK_BLKS, nblocks - b)
        out.append((b, nb))
        b += nb
    return out


def _spans(tpb, chunks):
    """Partition the tile axis into spans of <= SPAN_COLS cols aligned to
    chunk boundaries. Returns (tile_base, spans list[(t0,t1)], span_of_chunk)."""
    tile_base = [0]
    for t in tpb:
        tile_base.append(tile_base[-1] + t)
    spans = []
    span_of_chunk = []
    cur0 = 0
    for (b, nb) in chunks:
        t0, t1 = tile_base[b], tile_base[b + nb]
        if t1 - cur0 > SPAN_COLS and t0 > cur0:
            spans.append((cur0, t0))
            cur0 = t0
        span_of_chunk.append(len(spans))
    spans.append((cur0, tile_base[-1]))
    return tile_base, spans, span_of_chunk


def build(cfg, grids, b2val):
    """grids: dict rel -> (tile_base, tpb, spans, span_of_chunk, T_total)."""
    H, FP, FS, L = cfg.H, cfg.FP, cfg.FS, cfg.L
    nc = bacc.Bacc(None, target_bir_lowering=False, num_devices=C, num_swdge_queues=4)

    def param(name, shape, dt):
        return nc.declare_dram_parameter(name, shape, dt, isOutput=False)

    xpt = param("xpt", [FP, cfg.NPP], BF)
    xst = param("xst", [FS, cfg.NSP], BF)
    eparams = {}
    for r in ("orb", "hst", "sib"):
        T = grids[r][4]
        eparams[r] = (
            param(f"{r}_src", [BLK, T], I32),
            param(f"{r}_dr", [BLK, T], F32),
            param(f"{r}_w", [BLK, T], F32),
        )
    iota_p = param("iota", [128, 512], F32)
    wp_p = param("wp", [FP, H], BF)
    bp_p = param("bp", [H, 1], F32)
    ws_p = param("ws", [FS, H], BF)
    bs_p = param("bs", [H, 1], F32)
    wstack_s_p = param("wstack_s", [L, 2 * H, H], BF)
    wstack_p_p = param("wstack_p", [L, 2 * H, H], BF)
    wr_p_p = param("wr_p", [L, H, H], BF)
    bias_s_p = param("bias_s", [L, H, 1], F32)
    bias_p_p = param("bias_p", [L, H, 1], F32)
    w1_p = param("w1", [H, H // 2], BF)
    b1_p = param("b1", [H // 2, 1], F32)
    w2_p = param("w2", [H // 2, 1], BF)
    ident_p = param("ident", [H, H], BF)
    out_p = nc.declare_dram_parameter("out", [1, cfg.NPP], F32, isOutput=True)

    pchunks = _chunks(cfg.PB)
    schunks = _chunks(cfg.SB)

    with tile.TileContext(nc) as tc:
        with (
            tc.tile_pool(name="const", bufs=1) as cp,
            tc.tile_pool(name="dram", bufs=1, space="DRAM") as dp,
            tc.tile_pool(name="idx", bufs=2) as ip,
            tc.tile_pool(name="msg", bufs=3) as mp,
            tc.tile_pool(name="sel", bufs=16) as selp,
            tc.tile_pool(name="work", bufs=4) as wkp,
            tc.tile_pool(name="psum", bufs=1, space="PSUM") as pp,
        ):
            # ---- persistent DRAM state ----
            hp_tab = [
                dp.tile([cfg.NPT, H], BF, addr_space="Shared", tag=f"hp_tab{i}", name=f"hp_tab{i}")
                for i in range(L)
            ]
            hs_tab = [
                dp.tile([cfg.NST, H], BF, addr_space="Shared", tag=f"hs_tab{i}", name=f"hs_tab{i}")
                for i in range(L)
            ]
            xpT = [dp.tile([H, cfg.NPP], BF, tag=f"xpT{i}", name=f"xpT{i}") for i in range(2)]
            xsT = [dp.tile([H, cfg.NSP], BF, tag=f"xsT{i}", name=f"xsT{i}") for i in range(2)]
            hp_shard = dp.tile([cfg.NPP, H], BF, tag="hp_shard")
            hs_shard = dp.tile([cfg.NSP, H], BF, tag="hs_shard")

            # ---- consts ----
            iota_t = cp.tile([128, 512], F32, tag="iota")
            nc.sync.dma_start(out=iota_t[:], in_=iota_p[:, :])
            wp_t = cp.tile([FP, H], BF, tag="wp")
            nc.sync.dma_start(out=wp_t[:], in_=wp_p[:, :])
            ws_t = cp.tile([FS, H], BF, tag="ws")
            nc.sync.dma_start(out=ws_t[:], in_=ws_p[:, :])
            bp_t = cp.tile([H, 1], F32, tag="bp")
            nc.sync.dma_start(out=bp_t[:], in_=bp_p[:, :])
            bs_t = cp.tile([H, 1], F32, tag="bs")
            nc.sync.dma_start(out=bs_t[:], in_=bs_p[:, :])
            w1_t = cp.tile([H, H // 2], BF, tag="w1")
            nc.sync.dma_start(out=w1_t[:], in_=w1_p[:, :])
            b1_t = cp.tile([H // 2, 1], F32, tag="b1")
            nc.sync.dma_start(out=b1_t[:], in_=b1_p[:, :])
            w2_t = cp.tile([H // 2, 1], BF, tag="w2")
            nc.sync.dma_start(out=w2_t[:], in_=w2_p[:, :])
            ident_t = cp.tile([H, H], BF, tag="ident")
            nc.sync.dma_start(out=ident_t[:], in_=ident_p[:, :])
            wstack_s_t, wstack_p_t, wr_p_t, bias_s_t, bias_p_t = [], [], [], [], []
            for l in range(L):
                t = cp.tile([2 * H, H], BF, tag=f"wss{l}")
                nc.sync.dma_start(out=t[:], in_=wstack_s_p[l, :, :])
                wstack_s_t.append(t)
                t = cp.tile([2 * H, H], BF, tag=f"wsp{l}")
                nc.sync.dma_start(out=t[:], in_=wstack_p_p[l, :, :])
                wstack_p_t.append(t)
                t = cp.tile([H, H], BF, tag=f"wrp{l}")
                nc.sync.dma_start(out=t[:], in_=wr_p_p[l, :, :])
                wr_p_t.append(t)
                t = cp.tile([H, 1], F32, tag=f"bss{l}")
                nc.sync.dma_start(out=t[:], in_=bias_s_p[l, :, :])
                bias_s_t.append(t)
                t = cp.tile([H, 1], F32, tag=f"bsp{l}")
                nc.sync.dma_start(out=t[:], in_=bias_p_p[l, :, :])
                bias_p_t.append(t)

            def allgather(shard, tab):
                nc.gpsimd.collective_compute(
                    "AllGather",
                    ALU.bypass,
                    replica_groups=[list(range(C))],
                    ins=[shard[:, :]],
                    outs=[tab[:, :]],
                )

            def write_out_chunk(ob, c0, cw, nb, shard, xT_next):
                # TensorE transposes (engine otherwise idle) instead of xbar
                # DMA transposes: frees the sync/HWDGE queue, which serialized
                # the projection phase and competed with span/self loads.
                nc.sync.dma_start(out=xT_next[:, c0 : c0 + cw], in_=ob[:, :cw])
                pt = pp.tile([128, CHUNK_BLKS * H], BF, tag="wbt", bufs=2)
                for bi in range(nb):
                    nc.tensor.transpose(
                        out=pt[:, bi * H : (bi + 1) * H],
                        in_=ob[:, bi * 128 : (bi + 1) * 128],
                        identity=ident_t[:],
                    )
                nm = wkp.tile([128, CHUNK_BLKS * H], BF, tag="nm", bufs=4)
                nc.scalar.activation(
                    out=nm[:, : nb * H], in_=pt[:, : nb * H], func=AF.Copy
                )
                for bi in range(nb):
                    r0 = c0 + bi * 128
                    nc.sync.dma_start(
                        out=shard[r0 : r0 + 128, :],
                        in_=nm[:, bi * H : (bi + 1) * H],
                    )

            class SpanState:
                def __init__(self, rel):
                    self.rel = rel
                    self.cur = -1
                    self.tiles = None

                def ensure(self, si, spans):
                    if self.cur == si:
                        return
                    self.cur = si
                    t0, t1 = spans[si]
                    n = t1 - t0
                    sp, dp_, wp_ = eparams[self.rel]
                    st = ip.tile([BLK, n], I32, tag=f"{self.rel}_src")
                    nc.sync.dma_start(out=st[:], in_=sp[:, t0:t1])
                    dt_ = ip.tile([BLK, n], F32, tag=f"{self.rel}_dr")
                    nc.sync.dma_start(out=dt_[:], in_=dp_[:, t0:t1])
                    wt = ip.tile([BLK, n], F32, tag=f"{self.rel}_w")
                    nc.sync.dma_start(out=wt[:], in_=wp_[:, t0:t1])
                    self.tiles = (st, dt_, wt, t0)

            def agg_chunk(rel, state, table, b0, nb, ci, agg_psum):
                """Emit gathers + one-hot + segment matmuls for chunk [b0, b0+nb)
                of relation rel, accumulating aggT into agg_psum [64, nb*128].

                Each gather lands in its own small tile so the per-tile matmul
                only waits on its OWN gather (not the whole chunk's), keeping
                the Pool engine (the serialized descgen bottleneck) saturated."""
                tile_base, tpb, spans, soc, T = grids[rel]
                cw = nb * BLK
                state.ensure(soc[ci], spans)
                st, dt_, wt, t0 = state.tiles
                c_t0, c_t1 = tile_base[ci], tile_base[ci + 1]
                gk = c_t1 - c_t0
                msgs = []
                for j in range(gk):
                    t = c_t0 + j
                    jj = t - t0
                    qn = t % N_SWDGE_Q
                    m = mp.tile([128, H], BF, tag=f"{rel}_msg{j % MSG_TAGS}",
                                bufs=MSG_BUFS, name=f"{rel}_msgc")
                    nc.gpsimd.indirect_dma_start_q(
                        out=m[:],
                        out_offset=None,
                        in_=table[:, :],
                        in_offset=IndirectOffsetOnAxis(
                            ap=st[:, jj : jj + 1], axis=0
                        ),
                        queue=f"qPoolDynamic{qn or ''}",
                    )
                    msgs.append(m)
                for j in range(gk):
                    t = c_t0 + j
                    jj = t - t0
                    sel = selp.tile([128, cw], BF, tag="sel", name="sel")
                    nc.vector.tensor_scalar(
                        out=sel[:],
                        in0=iota_t[:, :cw],
                        scalar1=dt_[:, jj : jj + 1],
                        scalar2=wt[:, jj : jj + 1],
                        op0=ALU.is_equal,
                        op1=ALU.mult,
                    )
                    nc.tensor.matmul(
                        out=agg_psum[:, :cw],
                        lhsT=msgs[j][:],
                        rhs=sel[:],
                        start=(j == 0),
                        stop=(j == gk - 1),
                    )

            # =================== input projection ===================
            for (b0, nb) in pchunks:
                cw = nb * BLK
                c0 = b0 * BLK
                xp = wkp.tile([FP, cw], BF, tag="xp")
                nc.sync.dma_start(out=xp[:], in_=xpt[:, c0 : c0 + cw])
                po = pp.tile([H, cw], F32, tag="out", bufs=2)
                nc.tensor.matmul(
                    out=po[:], lhsT=wp_t[:], rhs=xp[:], start=True, stop=True
                )
                ob = wkp.tile([H, cw], BF, tag="ob")
                nc.scalar.activation(
                    out=ob[:], in_=po[:], func=AF.Relu, bias=bp_t[:], scale=1.0
                )
                write_out_chunk(ob, c0, cw, nb, hp_shard, xpT[0])
            for (b0, nb) in schunks:
                cw = nb * BLK
                c0 = b0 * BLK
                xs = wkp.tile([FS, cw], BF, tag="xs")
                nc.sync.dma_start(out=xs[:], in_=xst[:, c0 : c0 + cw])
                po = pp.tile([H, cw], F32, tag="out", bufs=2)
                nc.tensor.matmul(
                    out=po[:], lhsT=ws_t[:], rhs=xs[:], start=True, stop=True
                )
                ob = wkp.tile([H, cw], BF, tag="ob")
                nc.scalar.activation(
                    out=ob[:], in_=po[:], func=AF.Relu, bias=bs_t[:], scale=1.0
                )
                write_out_chunk(ob, c0, cw, nb, hs_shard, xsT[0])
            allgather(hp_shard, hp_tab[0])
            allgather(hs_shard, hs_tab[0])

            # =================== SAGE layers ===================
            # Planets first, then AG(hp) issued so it overlaps the star phase
            # (stars read layer-l tables only), then stars, then AG(hs).
            for l in range(L):
                rp, wpar = l % 2, (l + 1) % 2
                rv, wv = l, l + 1
                # ---- planets ----
                st_hst = SpanState("hst")
                st_sib = SpanState("sib")
                for ci, (b0, nb) in enumerate(pchunks):
                    cw = nb * BLK
                    c0 = b0 * BLK
                    agg_h = pp.tile([H, CHUNK_BLKS * BLK], F32, tag="agg_a", bufs=2)
                    agg_chunk("hst", st_hst, hs_tab[rv], b0, nb, ci, agg_h)
                    agg_s = pp.tile([H, CHUNK_BLKS * BLK], F32, tag="agg_b", bufs=2)
                    agg_chunk("sib", st_sib, hp_tab[rv], b0, nb, ci, agg_s)
                    stacked = wkp.tile([2 * H, cw], BF, tag="stacked")
                    nc.scalar.activation(
                        out=stacked[0:H, :], in_=agg_h[:, :cw], func=AF.Copy
                    )
                    nc.scalar.activation(
                        out=stacked[H : 2 * H, :], in_=agg_s[:, :cw], func=AF.Copy
                    )
                    xt = wkp.tile([H, cw], BF, tag="xt")
                    nc.sync.dma_start(out=xt[:], in_=xpT[rp][:, c0 : c0 + cw])
                    po = pp.tile([H, cw], F32, tag="out", bufs=2)
                    nc.tensor.matmul(
                        out=po[:],
                        lhsT=wstack_p_t[l][:],
                        rhs=stacked[:],
                        start=True,
                        stop=False,
                    )
                    nc.tensor.matmul(
                        out=po[:], lhsT=wr_p_t[l][:], rhs=xt[:],
                        start=False, stop=True,
                    )
                    ob = wkp.tile([H, cw], BF, tag="ob")
                    nc.scalar.activation(
                        out=ob[:], in_=po[:], func=AF.Relu,
                        bias=bias_p_t[l][:], scale=1.0,
                    )
                    if l < L - 1:
                        write_out_chunk(ob, c0, cw, nb, hp_shard, xpT[wpar])
                    else:
                        # fused readout: relu(ob^T W1 + b1) W2 + b2
                        # (readout psums reuse the "out" ring to stay in 8 banks)
                        prt = pp.tile([H, cw], F32, tag="out", bufs=2)
                        pr = prt[0 : H // 2, :]
                        nc.tensor.matmul(
                            out=pr, lhsT=w1_t[:], rhs=ob[:], start=True, stop=True
                        )
                        r1 = wkp.tile([H // 2, cw], BF, tag="r1sb")
                        nc.scalar.activation(
                            out=r1[:], in_=pr, func=AF.Relu,
                            bias=b1_t[:], scale=1.0,
                        )
                        pyt = pp.tile([H, cw], F32, tag="out", bufs=2)
                        py = pyt[0:1, :]
                        nc.tensor.matmul(
                            out=py, lhsT=w2_t[:], rhs=r1[:], start=True, stop=True
                        )
                        ysb = wkp.tile([1, cw], F32, tag="ysb")
                        nc.vector.tensor_scalar_add(
                            out=ysb[:], in0=py, scalar1=float(b2val)
                        )
                        nc.sync.dma_start(out=out_p[0:1, c0 : c0 + cw], in_=ysb[:])
                if l < L - 1:
                    allgather(hp_shard, hp_tab[wv])
                # ---- stars (skip at last layer: no consumer); overlaps AG(hp)
                if l < L - 1:
                    st_orb = SpanState("orb")
                    for ci, (b0, nb) in enumerate(schunks):
                        cw = nb * BLK
                        c0 = b0 * BLK
                        agg = pp.tile([H, CHUNK_BLKS * BLK], F32, tag="agg_a", bufs=2)
                        agg_chunk("orb", st_orb, hp_tab[rv], b0, nb, ci, agg)
                        stacked = wkp.tile([2 * H, cw], BF, tag="stacked")
                        nc.scalar.activation(
                            out=stacked[0:H, :], in_=agg[:, :cw], func=AF.Copy
                        )
                        nc.sync.dma_start(
                            out=stacked[H : 2 * H, :], in_=xsT[rp][:, c0 : c0 + cw]
                        )
                        po = pp.tile([H, cw], F32, tag="out", bufs=2)
                        nc.tensor.matmul(
                            out=po[:],
                            lhsT=wstack_s_t[l][:],
                            rhs=stacked[:],
                            start=True,
                            stop=True,
                        )
                        ob = wkp.tile([H, cw], BF, tag="ob")
                        nc.scalar.activation(
                            out=ob[:], in_=po[:], func=AF.Relu,
                            bias=bias_s_t[l][:], scale=1.0,
                        )
                        write_out_chunk(ob, c0, cw, nb, hs_shard, xsT[wpar])
                    allgather(hs_shard, hs_tab[wv])

    nc.finalize()
    return nc


def _prep_all(inputs, cfg):
    f32 = np.float32
    xp = np.asarray(inputs["x_planet"], f32)
    xs = np.asarray(inputs["x_star"], f32)
    Wp = np.asarray(inputs["Wp"], f32)
    bp = np.asarray(inputs["bp"], f32)
    Ws = np.asarray(inputs["Ws"], f32)
    bs = np.asarray(inputs["bs"], f32)
    Wl = np.asarray(inputs["Wl"], f32)
    bl = np.asarray(inputs["bl"], f32)
    Wr = np.asarray(inputs["Wr"], f32)
    W1 = np.asarray(inputs["W1"], f32)
    b1 = np.asarray(inputs["b1"], f32)
    W2 = np.asarray(inputs["W2"], f32)
    b2 = np.asarray(inputs["b2"], f32)

    orb = _prep_rel(inputs["orbits_src"], inputs["orbits_dst"],
                    cfg.SP, cfg.NPP, cfg.SS, cfg.SB)
    hst = _prep_rel(inputs["hosts_src"], inputs["hosts_dst"],
                    cfg.SS, cfg.NSP, cfg.SP, cfg.PB)
    sib = _prep_rel(inputs["sib_src"], inputs["sib_dst"],
                    cfg.SP, cfg.NPP, cfg.SP, cfg.PB)

    grids = {}
    for name, r in (("orb", orb), ("hst", hst), ("sib", sib)):
        tpb = r[3]
        nblocks = cfg.SB if name == "orb" else cfg.PB
        nsegs = len(tpb)
        seg_chunks = [(i, 1) for i in range(nsegs)]
        tile_base, spans, soc = _spans(tpb, seg_chunks)
        grids[name] = (tile_base, tpb, spans, soc, tile_base[-1])

    L, H = cfg.L, cfg.H
    wstack_s = np.stack([np.concatenate([Wl[l, 0], Wr[l, 0]], 0) for l in range(L)])
    wstack_p = np.stack(
        [np.concatenate([0.5 * Wl[l, 1], 0.5 * Wl[l, 2]], 0) for l in range(L)]
    )
    wr_p = np.stack([0.5 * (Wr[l, 1] + Wr[l, 2]) for l in range(L)])
    bias_s = np.stack([bl[l, 0][:, None] for l in range(L)])
    bias_p = np.stack([0.5 * (bl[l, 1] + bl[l, 2])[:, None] for l in range(L)])
    iota = np.tile(np.arange(512, dtype=np.float32), (128, 1))

    common = {
        "iota": iota,
        "wp": Wp.astype(BF16), "bp": bp[:, None],
        "ws": Ws.astype(BF16), "bs": bs[:, None],
        "wstack_s": wstack_s.astype(BF16), "wstack_p": wstack_p.astype(BF16),
        "wr_p": wr_p.astype(BF16),
        "bias_s": bias_s, "bias_p": bias_p,
        "w1": W1.astype(BF16), "b1": b1[:, None], "w2": W2.astype(BF16),
        "ident": np.eye(H, dtype=BF16),
    }
    in_maps = []
    for c in range(C):
        xpt_c = np.zeros((cfg.FP, cfg.NPP), BF16)
        xpt_c[:, : cfg.SP] = xp[c * cfg.SP : (c + 1) * cfg.SP].T.astype(BF16)
        xst_c = np.zeros((cfg.FS, cfg.NSP), BF16)
        xst_c[:, : cfg.SS] = xs[c * cfg.SS : (c + 1) * cfg.SS].T.astype(BF16)
        m = dict(common)
        m["xpt"] = xpt_c
        m["xst"] = xst_c
        for name, r in (("orb", orb), ("hst", hst), ("sib", sib)):
            m[f"{name}_src"] = r[0][c]
            m[f"{name}_dr"] = r[1][c]
            m[f"{name}_w"] = r[2][c]
        in_maps.append(m)
    return in_maps, grids, float(b2[0])


LAST_RESULT = None


def kernel(_cfg=None, _trace=False, **inputs):
    global LAST_RESULT
    cfg = _cfg or Cfg()
    in_maps, grids, b2val = _prep_all(inputs, cfg)
    nc = build(cfg, grids, b2val)
    res = run_bass_kernel_spmd(nc, in_maps, list(range(C)), trace=_trace)
    LAST_RESULT = res
    out = np.concatenate(
        [res.results[c]["out"][0, : cfg.SP] for c in range(C)]
    ).astype(np.float32)
    return out



# revision 12
# speedup vs baseline: 1.4622x; 1.0429x over previous
"""Trainium2 Bass kernel for the ExoplanetGNN heterograph message-passing net.

Self-contained: builds host-side edge grids, compiles one SPMD Bass program,
runs it on 8 NeuronCores via run_bass_kernel_spmd, reassembles the output.

Design:
 - dst-sharded edges: core c owns planet shard c and star shard c and all edges
   whose dst lands there; aggregation is complete per core (no reduce).
 - node feature tables (bf16, node-major [rows, 64]) are fully replicated per
   core; after each layer, shards are AllGathered into the next layer's table.
 - per-128-edge tile: indirect-DMA row gather ([128,1] offsets), DVE one-hot
   (iota is_equal dst_rel) * (1/deg), TensorE segment matmul accumulating
   transposed aggregates [64 feat x 128 nodes] in PSUM.
 - per 512-node chunk: stacked [128, 512] rhs (two relations' aggregates for
   planets / aggregate+xT for stars), one or two K=128/64 matmuls apply the
   SAGE linear layers, ScalarE fuses bias+ReLU, HWDGE dma-transpose produces
   node-major tiles for the table shard; a feat-major copy (xT) is kept for
   the next layer's self term.
 - layer 2 skips the star update and fuses the readout MLP on the planet path.
"""

import math

import numpy as np
import ml_dtypes

import concourse.bass as bass
import concourse.bacc as bacc
import concourse.mybir as mybir
import concourse.tile as tile
from concourse.bass import IndirectOffsetOnAxis
from concourse.bass_utils import run_bass_kernel_spmd

BF16 = ml_dtypes.bfloat16
BF = mybir.dt.bfloat16
F32 = mybir.dt.float32
I32 = mybir.dt.int32
AF = mybir.ActivationFunctionType
ALU = mybir.AluOpType

C = 8          # cores
N_SWDGE_Q = 4
BLK = 128      # dst nodes per block
CHUNK_BLKS = 4 # node blocks per compute chunk
SPAN_COLS = 512  # max index columns per span load
MSG_TAGS = 16  # distinct per-gather msg buffer tags
MSG_BUFS = 3   # ring depth per tag (48 gathers in flight)




def _patch_indirect_queue():
    """Recompile BassGpSimd.indirect_dma_start with a queue= parameter."""
    import inspect, textwrap, re
    src_ = textwrap.dedent(inspect.getsource(bass.BassGpSimd.indirect_dma_start))
    src_ = src_.replace("def indirect_dma_start(", "def indirect_dma_start_q(")
    src_ = src_.replace("compute_op: mybir.AluOpType = mybir.AluOpType.bypass,",
                        "compute_op: mybir.AluOpType = mybir.AluOpType.bypass, queue: str = \"qPoolDynamic\",")
    src_ = src_.replace('queue="qPoolDynamic"', "queue=queue")
    ns = vars(bass).copy()
    exec(compile(src_, "<indirect_q>", "exec"), ns)
    bass.BassGpSimd.indirect_dma_start_q = ns["indirect_dma_start_q"]


_patch_indirect_queue()


class Cfg:
    def __init__(self, np_=500000, ns_=200000, fp=32, fs=16, h=64, l=3):
        self.NP, self.NS, self.FP, self.FS, self.H, self.L = np_, ns_, fp, fs, h, l
        assert np_ % C == 0 and ns_ % C == 0
        self.SP, self.SS = np_ // C, ns_ // C
        self.PB = -(-self.SP // BLK)
        self.SB = -(-self.SS // BLK)
        self.NPP, self.NSP = self.PB * BLK, self.SB * BLK
        self.NPT, self.NST = C * self.NPP, C * self.NSP


def _prep_rel(src, dst, src_shard, src_pad, dst_shard, dst_blocks, seg=512):
    """Build per-core tile-transposed edge arrays for one relation.

    Returns (srcT [C,128,T] int32, dr [C,128,T] f32, w [C,128,T] f32,
    tiles_per_block list[int] of len dst_blocks).
    Slot (t, p) holds edge i = <pos p of tile t>; tile t belongs to one dst
    block; pad slots: src=0, dr=-1, w=0.
    """
    src = np.asarray(src, np.int64)
    dst = np.asarray(dst, np.int64)
    core = dst // dst_shard
    loc = dst - core * dst_shard
    blk = loc // seg
    rel = loc - blk * seg
    PB = -(-(dst_blocks * BLK) // seg)
    key = core * PB + blk
    cnt = np.bincount(key, minlength=C * PB).reshape(C, PB)
    tpb = np.maximum(1, -(-cnt.max(axis=0) // BLK))
    tile_base = np.concatenate([[0], np.cumsum(tpb)]).astype(np.int64)
    T = int(tile_base[-1])

    order = np.argsort(key, kind="stable")
    key_s = key[order]
    firsts = np.searchsorted(key_s, np.arange(C * PB))
    pos = np.arange(len(key_s)) - firsts[key_s]
    blk_s = blk[order]
    t_idx = tile_base[blk_s] + pos // BLK
    p_idx = pos % BLK
    c_idx = core[order]

    deg = np.bincount(core * dst_shard + loc, minlength=C * dst_shard).astype(np.float64)
    w_e = (1.0 / np.maximum(deg, 1.0))[core * dst_shard + loc]

    s_core = src // src_shard
    s_pad = s_core * src_pad + (src - s_core * src_shard)

    srcT = np.zeros((C, BLK, T), np.int32)
    dr = np.full((C, BLK, T), -1.0, np.float32)
    w = np.zeros((C, BLK, T), np.float32)
    srcT[c_idx, p_idx, t_idx] = s_pad[order]
    dr[c_idx, p_idx, t_idx] = rel[order]
    w[c_idx, p_idx, t_idx] = w_e[order]
    return srcT, dr, w, tpb.tolist()


def _chunks(nblocks):
    out = []
    b = 0
    while b < nblocks:
        nb = min(CHUNK_BLKS, nblocks - b)
        out.append((b, nb))
        b += nb
    return out


def _spans(tpb, chunks):
    """Partition the tile axis into spans of <= SPAN_COLS cols aligned to
    chunk boundaries. Returns (tile_base, spans list[(t0,t1)], span_of_chunk)."""
    tile_base = [0]
    for t in tpb:
        tile_base.append(tile_base[-1] + t)
    spans = []
    span_of_chunk = []
    cur0 = 0
    for (b, nb) in chunks:
        t0, t1 = tile_base[b], tile_base[b + nb]
        if t1 - cur0 > SPAN_COLS and t0 > cur0:
            spans.append((cur0, t0))
            cur0 = t0
        span_of_chunk.append(len(spans))
    spans.append((cur0, tile_base[-1]))
    return tile_base, spans, span_of_chunk


def build(cfg, grids, b2val):
    """grids: dict rel -> (tile_base, tpb, spans, span_of_chunk, T_total)."""
    H, FP, FS, L = cfg.H, cfg.FP, cfg.FS, cfg.L
    nc = bacc.Bacc(None, target_bir_lowering=False, num_devices=C, num_swdge_queues=4)

    def param(name, shape, dt):
        return nc.declare_dram_parameter(name, shape, dt, isOutput=False)

    xpt = param("xpt", [FP, cfg.NPP], BF)
    xst = param("xst", [FS, cfg.NSP], BF)
    eparams = {}
    for r in ("orb", "hst", "sib"):
        T = grids[r][4]
        eparams[r] = (
            param(f"{r}_src", [BLK, T], I32),
            param(f"{r}_dr", [BLK, T], F32),
            param(f"{r}_w", [BLK, T], F32),
        )
    iota_p = param("iota", [128, 512], F32)
    wp_p = param("wp", [FP, H], BF)
    bp_p = param("bp", [H, 1], F32)
    ws_p = param("ws", [FS, H], BF)
    bs_p = param("bs", [H, 1], F32)
    wstack_s_p = param("wstack_s", [L, 2 * H, H], BF)
    wstack_p_p = param("wstack_p", [L, 2 * H, H], BF)
    wr_p_p = param("wr_p", [L, H, H], BF)
    bias_s_p = param("bias_s", [L, H, 1], F32)
    bias_p_p = param("bias_p", [L, H, 1], F32)
    w1_p = param("w1", [H, H // 2], BF)
    b1_p = param("b1", [H // 2, 1], F32)
    w2_p = param("w2", [H // 2, 1], BF)
    ident_p = param("ident", [H, H], BF)
    out_p = nc.declare_dram_parameter("out", [1, cfg.NPP], F32, isOutput=True)

    pchunks = _chunks(cfg.PB)
    schunks = _chunks(cfg.SB)

    with tile.TileContext(nc) as tc:
        with (
            tc.tile_pool(name="const", bufs=1) as cp,
            tc.tile_pool(name="dram", bufs=1, space="DRAM") as dp,
            tc.tile_pool(name="idx", bufs=2) as ip,
            tc.tile_pool(name="msg", bufs=3) as mp,
            tc.tile_pool(name="sel", bufs=16) as selp,
            tc.tile_pool(name="work", bufs=4) as wkp,
            tc.tile_pool(name="psum", bufs=1, space="PSUM") as pp,
        ):
            # ---- persistent DRAM state ----
            hp_tab = [
                dp.tile([cfg.NPT, H], BF, addr_space="Shared", tag=f"hp_tab{i}", name=f"hp_tab{i}")
                for i in range(L)
            ]
            hs_tab = [
                dp.tile([cfg.NST, H], BF, addr_space="Shared", tag=f"hs_tab{i}", name=f"hs_tab{i}")
                for i in range(L)
            ]
            xpT = [dp.tile([H, cfg.NPP], BF, tag=f"xpT{i}", name=f"xpT{i}") for i in range(2)]
            xsT = [dp.tile([H, cfg.NSP], BF, tag=f"xsT{i}", name=f"xsT{i}") for i in range(2)]
            hp_shard = dp.tile([cfg.NPP, H], BF, tag="hp_shard")
            hs_shard = dp.tile([cfg.NSP, H], BF, tag="hs_shard")

            # ---- consts ----
            iota_t = cp.tile([128, 512], F32, tag="iota")
            nc.sync.dma_start(out=iota_t[:], in_=iota_p[:, :])
            wp_t = cp.tile([FP, H], BF, tag="wp")
            nc.sync.dma_start(out=wp_t[:], in_=wp_p[:, :])
            ws_t = cp.tile([FS, H], BF, tag="ws")
            nc.sync.dma_start(out=ws_t[:], in_=ws_p[:, :])
            bp_t = cp.tile([H, 1], F32, tag="bp")
            nc.sync.dma_start(out=bp_t[:], in_=bp_p[:, :])
            bs_t = cp.tile([H, 1], F32, tag="bs")
            nc.sync.dma_start(out=bs_t[:], in_=bs_p[:, :])
            w1_t = cp.tile([H, H // 2], BF, tag="w1")
            nc.sync.dma_start(out=w1_t[:], in_=w1_p[:, :])
            b1_t = cp.tile([H // 2, 1], F32, tag="b1")
            nc.sync.dma_start(out=b1_t[:], in_=b1_p[:, :])
            w2_t = cp.tile([H // 2, 1], BF, tag="w2")
            nc.sync.dma_start(out=w2_t[:], in_=w2_p[:, :])
            ident_t = cp.tile([H, H], BF, tag="ident")
            nc.sync.dma_start(out=ident_t[:], in_=ident_p[:, :])
            wstack_s_t, wstack_p_t, wr_p_t, bias_s_t, bias_p_t = [], [], [], [], []
            for l in range(L):
                t = cp.tile([2 * H, H], BF, tag=f"wss{l}")
                nc.sync.dma_start(out=t[:], in_=wstack_s_p[l, :, :])
                wstack_s_t.append(t)
                t = cp.tile([2 * H, H], BF, tag=f"wsp{l}")
                nc.sync.dma_start(out=t[:], in_=wstack_p_p[l, :, :])
                wstack_p_t.append(t)
                t = cp.tile([H, H], BF, tag=f"wrp{l}")
                nc.sync.dma_start(out=t[:], in_=wr_p_p[l, :, :])
                wr_p_t.append(t)
                t = cp.tile([H, 1], F32, tag=f"bss{l}")
                nc.sync.dma_start(out=t[:], in_=bias_s_p[l, :, :])
                bias_s_t.append(t)
                t = cp.tile([H, 1], F32, tag=f"bsp{l}")
                nc.sync.dma_start(out=t[:], in_=bias_p_p[l, :, :])
                bias_p_t.append(t)

            def allgather(shard, tab):
                nc.gpsimd.collective_compute(
                    "AllGather",
                    ALU.bypass,
                    replica_groups=[list(range(C))],
                    ins=[shard[:, :]],
                    outs=[tab[:, :]],
                )

            def write_out_chunk(ob, c0, cw, nb, shard, xT_next):
                # TensorE transposes (engine otherwise idle) instead of xbar
                # DMA transposes: frees the sync/HWDGE queue, which serialized
                # the projection phase and competed with span/self loads.
                nc.sync.dma_start(out=xT_next[:, c0 : c0 + cw], in_=ob[:, :cw])
                pt = pp.tile([128, CHUNK_BLKS * H], BF, tag="wbt", bufs=2)
                for bi in range(nb):
                    nc.tensor.transpose(
                        out=pt[:, bi * H : (bi + 1) * H],
                        in_=ob[:, bi * 128 : (bi + 1) * 128],
                        identity=ident_t[:],
                    )
                nm = wkp.tile([128, CHUNK_BLKS * H], BF, tag="nm", bufs=4)
                nc.scalar.activation(
                    out=nm[:, : nb * H], in_=pt[:, : nb * H], func=AF.Copy
                )
                for bi in range(nb):
                    r0 = c0 + bi * 128
                    nc.sync.dma_start(
                        out=shard[r0 : r0 + 128, :],
                        in_=nm[:, bi * H : (bi + 1) * H],
                    )

            class SpanState:
                def __init__(self, rel):
                    self.rel = rel
                    self.cur = -1
                    self.tiles = None

                def ensure(self, si, spans):
                    if self.cur == si:
                        return
                    self.cur = si
                    t0, t1 = spans[si]
                    n = t1 - t0
                    sp, dp_, wp_ = eparams[self.rel]
                    st = ip.tile([BLK, n], I32, tag=f"{self.rel}_src")
                    nc.sync.dma_start(out=st[:], in_=sp[:, t0:t1])
                    dt_ = ip.tile([BLK, n], F32, tag=f"{self.rel}_dr")
                    nc.sync.dma_start(out=dt_[:], in_=dp_[:, t0:t1])
                    wt = ip.tile([BLK, n], F32, tag=f"{self.rel}_w")
                    nc.sync.dma_start(out=wt[:], in_=wp_[:, t0:t1])
                    self.tiles = (st, dt_, wt, t0)

            def agg_chunk(rel, state, table, b0, nb, ci, agg_psum):
                """Emit gathers + one-hot + segment matmuls for chunk [b0, b0+nb)
                of relation rel, accumulating aggT into agg_psum [64, nb*128].

                Each gather lands in its own small tile so the per-tile matmul
                only waits on its OWN gather (not the whole chunk's), keeping
                the Pool engine (the serialized descgen bottleneck) saturated."""
                tile_base, tpb, spans, soc, T = grids[rel]
                cw = nb * BLK
                state.ensure(soc[ci], spans)
                st, dt_, wt, t0 = state.tiles
                c_t0, c_t1 = tile_base[ci], tile_base[ci + 1]
                gk = c_t1 - c_t0
                msgs = []
                for j in range(gk):
                    t = c_t0 + j
                    jj = t - t0
                    qn = t % N_SWDGE_Q
                    m = mp.tile([128, H], BF, tag=f"{rel}_msg{j % MSG_TAGS}",
                                bufs=MSG_BUFS, name=f"{rel}_msgc")
                    nc.gpsimd.indirect_dma_start_q(
                        out=m[:],
                        out_offset=None,
                        in_=table[:, :],
                        in_offset=IndirectOffsetOnAxis(
                            ap=st[:, jj : jj + 1], axis=0
                        ),
                        queue=f"qPoolDynamic{qn or ''}",
                    )
                    msgs.append(m)
                for j in range(gk):
                    t = c_t0 + j
                    jj = t - t0
                    sel = selp.tile([128, cw], BF, tag="sel", name="sel")
                    nc.vector.tensor_scalar(
                        out=sel[:],
                        in0=iota_t[:, :cw],
                        scalar1=dt_[:, jj : jj + 1],
                        scalar2=wt[:, jj : jj + 1],
                        op0=ALU.is_equal,
                        op1=ALU.mult,
                    )
                    nc.tensor.matmul(
                        out=agg_psum[:, :cw],
                        lhsT=msgs[j][:],
                        rhs=sel[:],
                        start=(j == 0),
                        stop=(j == gk - 1),
                    )

            # =================== input projection ===================
            for (b0, nb) in pchunks:
                cw = nb * BLK
                c0 = b0 * BLK
                xp = wkp.tile([FP, cw], BF, tag="xp")
                nc.sync.dma_start(out=xp[:], in_=xpt[:, c0 : c0 + cw])
                po = pp.tile([H, cw], F32, tag="out", bufs=2)
                nc.tensor.matmul(
                    out=po[:], lhsT=wp_t[:], rhs=xp[:], start=True, stop=True
                )
                ob = wkp.tile([H, cw], BF, tag="ob")
                nc.scalar.activation(
                    out=ob[:], in_=po[:], func=AF.Relu, bias=bp_t[:], scale=1.0
                )
                write_out_chunk(ob, c0, cw, nb, hp_shard, xpT[0])
            for (b0, nb) in schunks:
                cw = nb * BLK
                c0 = b0 * BLK
                xs = wkp.tile([FS, cw], BF, tag="xs")
                nc.sync.dma_start(out=xs[:], in_=xst[:, c0 : c0 + cw])
                po = pp.tile([H, cw], F32, tag="out", bufs=2)
                nc.tensor.matmul(
                    out=po[:], lhsT=ws_t[:], rhs=xs[:], start=True, stop=True
                )
                ob = wkp.tile([H, cw], BF, tag="ob")
                nc.scalar.activation(
                    out=ob[:], in_=po[:], func=AF.Relu, bias=bs_t[:], scale=1.0
                )
                write_out_chunk(ob, c0, cw, nb, hs_shard, xsT[0])
            allgather(hp_shard, hp_tab[0])
            allgather(hs_shard, hs_tab[0])

            # =================== SAGE layers ===================
            # Planets first, then AG(hp) issued so it overlaps the star phase
            # (stars read layer-l tables only), then stars, then AG(hs).
            for l in range(L):
                rp, wpar = l % 2, (l + 1) % 2
                rv, wv = l, l + 1
                # ---- planets ----
                st_hst = SpanState("hst")
                st_sib = SpanState("sib")
                for ci, (b0, nb) in enumerate(pchunks):
                    cw = nb * BLK
                    c0 = b0 * BLK
                    agg_h = pp.tile([H, CHUNK_BLKS * BLK], F32, tag="agg_a", bufs=2)
                    agg_chunk("hst", st_hst, hs_tab[rv], b0, nb, ci, agg_h)
                    agg_s = pp.tile([H, CHUNK_BLKS * BLK], F32, tag="agg_b", bufs=2)
                    agg_chunk("sib", st_sib, hp_tab[rv], b0, nb, ci, agg_s)
                    stacked = wkp.tile([2 * H, cw], BF, tag="stacked")
                    nc.scalar.activation(
                        out=stacked[0:H, :], in_=agg_h[:, :cw], func=AF.Copy
                    )
                    nc.scalar.activation(
                        out=stacked[H : 2 * H, :], in_=agg_s[:, :cw], func=AF.Copy
                    )
                    xt = wkp.tile([H, cw], BF, tag="xt")
                    nc.sync.dma_start(out=xt[:], in_=xpT[rp][:, c0 : c0 + cw])
                    po = pp.tile([H, cw], F32, tag="out", bufs=2)
                    nc.tensor.matmul(
                        out=po[:],
                        lhsT=wstack_p_t[l][:],
                        rhs=stacked[:],
                        start=True,
                        stop=False,
                    )
                    nc.tensor.matmul(
                        out=po[:], lhsT=wr_p_t[l][:], rhs=xt[:],
                        start=False, stop=True,
                    )
                    ob = wkp.tile([H, cw], BF, tag="ob")
                    nc.scalar.activation(
                        out=ob[:], in_=po[:], func=AF.Relu,
                        bias=bias_p_t[l][:], scale=1.0,
                    )
                    if l < L - 1:
                        write_out_chunk(ob, c0, cw, nb, hp_shard, xpT[wpar])
                    else:
                        # fused readout: relu(ob^T W1 + b1) W2 + b2
                        # (readout psums reuse the "out" ring to stay in 8 banks)
                        prt = pp.tile([H, cw], F32, tag="out", bufs=2)
                        pr = prt[0 : H // 2, :]
                        nc.tensor.matmul(
                            out=pr, lhsT=w1_t[:], rhs=ob[:], start=True, stop=True
                        )
                        r1 = wkp.tile([H // 2, cw], BF, tag="r1sb")
                        nc.scalar.activation(
                            out=r1[:], in_=pr, func=AF.Relu,
                            bias=b1_t[:], scale=1.0,
                        )
                        pyt = pp.tile([H, cw], F32, tag="out", bufs=2)
                        py = pyt[0:1, :]
                        nc.tensor.matmul(
                            out=py, lhsT=w2_t[:], rhs=r1[:], start=True, stop=True
                        )
                        ysb = wkp.tile([1, cw], F32, tag="ysb")
                        nc.vector.tensor_scalar_add(
                            out=ysb[:], in0=py, scalar1=float(b2val)
                        )
                        nc.sync.dma_start(out=out_p[0:1, c0 : c0 + cw], in_=ysb[:])
                if l < L - 1:
                    allgather(hp_shard, hp_tab[wv])
                # ---- stars (skip at last layer: no consumer); overlaps AG(hp)
                if l < L - 1:
                    st_orb = SpanState("orb")
                    for ci, (b0, nb) in enumerate(schunks):
                        cw = nb * BLK
                        c0 = b0 * BLK
                        agg = pp.tile([H, CHUNK_BLKS * BLK], F32, tag="agg_a", bufs=2)
                        agg_chunk("orb", st_orb, hp_tab[rv], b0, nb, ci, agg)
                        stacked = wkp.tile([2 * H, cw], BF, tag="stacked")
                        nc.scalar.activation(
                            out=stacked[0:H, :], in_=agg[:, :cw], func=AF.Copy
                        )
                        nc.sync.dma_start(
                            out=stacked[H : 2 * H, :], in_=xsT[rp][:, c0 : c0 + cw]
                        )
                        po = pp.tile([H, cw], F32, tag="out", bufs=2)
                        nc.tensor.matmul(
                            out=po[:],
                            lhsT=wstack_s_t[l][:],
                            rhs=stacked[:],
                            start=True,
                            stop=True,
                        )
                        ob = wkp.tile([H, cw], BF, tag="ob")
                        nc.scalar.activation(
                            out=ob[:], in_=po[:], func=AF.Relu,
                            bias=bias_s_t[l][:], scale=1.0,
                        )
                        write_out_chunk(ob, c0, cw, nb, hs_shard, xsT[wpar])
                    allgather(hs_shard, hs_tab[wv])

    nc.finalize()
    return nc


def _degree_perm(deg_keys, n, shard):
    """Per-core-shard relabel: sort nodes by degree keys so every core's
    k-th 512-segment holds the same degree slice -> per-segment edge counts
    align across cores and the max-over-cores tile padding vanishes.
    Returns perm: old global id -> new global id (stays within its shard)."""
    perm = np.empty(n, np.int64)
    for c in range(n // shard):
        sl = slice(c * shard, (c + 1) * shard)
        order = np.lexsort(tuple(k[sl] for k in reversed(deg_keys)))
        rank = np.empty(shard, np.int64)
        rank[order] = np.arange(shard)
        perm[sl] = c * shard + rank
    return perm


def _prep_all(inputs, cfg):
    f32 = np.float32
    xp = np.asarray(inputs["x_planet"], f32)
    xs = np.asarray(inputs["x_star"], f32)
    Wp = np.asarray(inputs["Wp"], f32)
    bp = np.asarray(inputs["bp"], f32)
    Ws = np.asarray(inputs["Ws"], f32)
    bs = np.asarray(inputs["bs"], f32)
    Wl = np.asarray(inputs["Wl"], f32)
    bl = np.asarray(inputs["bl"], f32)
    Wr = np.asarray(inputs["Wr"], f32)
    W1 = np.asarray(inputs["W1"], f32)
    b1 = np.asarray(inputs["b1"], f32)
    W2 = np.asarray(inputs["W2"], f32)
    b2 = np.asarray(inputs["b2"], f32)

    i64 = np.int64
    orb_s = np.asarray(inputs["orbits_src"], i64)
    orb_d = np.asarray(inputs["orbits_dst"], i64)
    hst_s = np.asarray(inputs["hosts_src"], i64)
    hst_d = np.asarray(inputs["hosts_dst"], i64)
    sib_s = np.asarray(inputs["sib_src"], i64)
    sib_d = np.asarray(inputs["sib_dst"], i64)

    # Degree-sorted per-shard relabel: equalizes per-segment edge counts
    # across cores so the max-over-cores tile padding mostly disappears.
    sibdeg = np.bincount(sib_d, minlength=cfg.NP)
    hstdeg = np.bincount(hst_d, minlength=cfg.NP)
    orbdeg = np.bincount(orb_d, minlength=cfg.NS)
    pperm = _degree_perm([sibdeg, hstdeg], cfg.NP, cfg.SP)
    sperm = _degree_perm([orbdeg], cfg.NS, cfg.SS)
    xp2 = np.empty_like(xp)
    xp2[pperm] = xp
    xs2 = np.empty_like(xs)
    xs2[sperm] = xs
    xp, xs = xp2, xs2

    orb = _prep_rel(pperm[orb_s], sperm[orb_d],
                    cfg.SP, cfg.NPP, cfg.SS, cfg.SB)
    hst = _prep_rel(sperm[hst_s], pperm[hst_d],
                    cfg.SS, cfg.NSP, cfg.SP, cfg.PB)
    sib = _prep_rel(pperm[sib_s], pperm[sib_d],
                    cfg.SP, cfg.NPP, cfg.SP, cfg.PB)

    grids = {}
    for name, r in (("orb", orb), ("hst", hst), ("sib", sib)):
        tpb = r[3]
        nblocks = cfg.SB if name == "orb" else cfg.PB
        nsegs = len(tpb)
        seg_chunks = [(i, 1) for i in range(nsegs)]
        tile_base, spans, soc = _spans(tpb, seg_chunks)
        grids[name] = (tile_base, tpb, spans, soc, tile_base[-1])

    L, H = cfg.L, cfg.H
    wstack_s = np.stack([np.concatenate([Wl[l, 0], Wr[l, 0]], 0) for l in range(L)])
    wstack_p = np.stack(
        [np.concatenate([0.5 * Wl[l, 1], 0.5 * Wl[l, 2]], 0) for l in range(L)]
    )
    wr_p = np.stack([0.5 * (Wr[l, 1] + Wr[l, 2]) for l in range(L)])
    bias_s = np.stack([bl[l, 0][:, None] for l in range(L)])
    bias_p = np.stack([0.5 * (bl[l, 1] + bl[l, 2])[:, None] for l in range(L)])
    iota = np.tile(np.arange(512, dtype=np.float32), (128, 1))

    common = {
        "iota": iota,
        "wp": Wp.astype(BF16), "bp": bp[:, None],
        "ws": Ws.astype(BF16), "bs": bs[:, None],
        "wstack_s": wstack_s.astype(BF16), "wstack_p": wstack_p.astype(BF16),
        "wr_p": wr_p.astype(BF16),
        "bias_s": bias_s, "bias_p": bias_p,
        "w1": W1.astype(BF16), "b1": b1[:, None], "w2": W2.astype(BF16),
        "ident": np.eye(H, dtype=BF16),
    }
    in_maps = []
    for c in range(C):
        xpt_c = np.zeros((cfg.FP, cfg.NPP), BF16)
        xpt_c[:, : cfg.SP] = xp[c * cfg.SP : (c + 1) * cfg.SP].T.astype(BF16)
        xst_c = np.zeros((cfg.FS, cfg.NSP), BF16)
        xst_c[:, : cfg.SS] = xs[c * cfg.SS : (c + 1) * cfg.SS].T.astype(BF16)
        m = dict(common)
        m["xpt"] = xpt_c
        m["xst"] = xst_c
        for name, r in (("orb", orb), ("hst", hst), ("sib", sib)):
            m[f"{name}_src"] = r[0][c]
            m[f"{name}_dr"] = r[1][c]
            m[f"{name}_w"] = r[2][c]
        in_maps.append(m)
    return in_maps, grids, float(b2[0]), pperm


LAST_RESULT = None


def kernel(_cfg=None, _trace=False, **inputs):
    global LAST_RESULT
    cfg = _cfg or Cfg()
    in_maps, grids, b2val, pperm = _prep_all(inputs, cfg)
    nc = build(cfg, grids, b2val)
    res = run_bass_kernel_spmd(nc, in_maps, list(range(C)), trace=_trace)
    LAST_RESULT = res
    full_new = np.concatenate(
        [res.results[c]["out"][0, : cfg.SP] for c in range(C)]
    ).astype(np.float32)
    return full_new[pperm]

